# revision 14
# baseline (speedup 1.0000x reference)
"""Multi-head attention (B=4, S=2048, D=1024, H=16, causal) on 8 NeuronCores.

Sharding: data-parallel over batch (4) x tensor-parallel over head halves (2).
Core c handles batch c//2 with heads (c%2)*8 .. (c%2)*8+7 and produces output
columns (c%2)*512 .. +512 after an in-pair AllGather of the attention output.

Device pipeline (all bf16 matmuls, fp32 PSUM accumulation):
  phase 1: K and V projections from host-pre-transposed inputs -> KT [i, s]
           per head-pair, V1 [s, (V|ones)] per head with a 64-wide ones block
           so the PV matmul produces softmax denominators on 64 partitions.
  phase 2: per round r: Q projection chunk r (-> QT[r]) immediately followed
           by attention for head pair r, so ScalarE exp work starts ~70us
           earlier and overlaps the remaining projections.  Per head pair /
           512-query block: logits^T tiles [sk=128, sq<=512] via K=64
           row-packed matmul pairs (2 heads concurrent on the PE), exp on
           ScalarE (scale=1/8 fused), causal handled by restricting computed
           ranges + a triangular-mask multiply on the diagonal 128x128 tile,
           PV matmul with full M=128 (64 value cols + 64 ones cols).
           Softmax division uses vector.reciprocal_approx_fast (~5x faster
           than the iterative DVE reciprocal, ~18 correct bits).
  phase 3: pair-wise AllGather of attnT (issued per-pair, overlapped), then
           the output projection for this core's 512 output columns.

Biases are folded in only when nonzero (they are all zero for this model's
inputs): bq/bk via per-partition activation bias on the projection drains,
bv/bo via partition-broadcast tiles added on the V1/output drains.  No PE
cycles are ever spent on biases.
"""

import numpy as np
import ml_dtypes

import concourse.bass as bass
import concourse.mybir as mybir
import concourse.tile as tile
from concourse import bacc
from concourse.bass_utils import run_bass_kernel_spmd

B, S, D, H = 4, 2048, 1024, 16
HD = D // H  # 64
NCORES = 8
HH = D // 2  # 512 = head-half width (8 heads x 64) = output col split
BF16 = mybir.dt.bfloat16
F32 = mybir.dt.float32
NPBF = ml_dtypes.bfloat16

P = 128          # partitions
NB = S // 512    # 4 query/seq blocks of 512
NT = S // P      # 16 seq tiles of 128
NC = D // P      # 8 contraction chunks of 128
NPAIR = 4        # head pairs per core

REPLICA_GROUPS = [[0, 1], [2, 3], [4, 5], [6, 7]]

_cache = {}


def _build(has_bq, has_bk, has_bv, has_bo):
    nc = bacc.Bacc("TRN2", target_bir_lowering=False, debug=False,
                   num_devices=NCORES)

    # ---- dram I/O ----
    qT = nc.dram_tensor("qT", [P, NC, S], BF16, kind="ExternalInput")
    kT = nc.dram_tensor("kT", [P, NC, S], BF16, kind="ExternalInput")
    vT = nc.dram_tensor("vT", [P, NC, S], BF16, kind="ExternalInput")
    wqT = nc.dram_tensor("wqT", [P, NC, HH], BF16, kind="ExternalInput")
    wkT = nc.dram_tensor("wkT", [P, NC, HH], BF16, kind="ExternalInput")
    wvT = nc.dram_tensor("wvT", [P, NC, HH], BF16, kind="ExternalInput")
    woT = nc.dram_tensor("woT", [P, NC, HH], BF16, kind="ExternalInput")
    bqv = nc.dram_tensor("bq", [P, 4], F32, kind="ExternalInput")
    bkv = nc.dram_tensor("bk", [P, 4], F32, kind="ExternalInput")
    bvv = nc.dram_tensor("bv", [1, HH], F32, kind="ExternalInput")
    bov = nc.dram_tensor("bo", [1, HH], F32, kind="ExternalInput")
    trid = nc.dram_tensor("tri", [P, P], BF16, kind="ExternalInput")
    out = nc.dram_tensor("out", [S, HH], F32, kind="ExternalOutput")

    ID = mybir.ActivationFunctionType.Identity

    with tile.TileContext(nc) as tc:
        with (
            tc.tile_pool(name="persist", bufs=1) as pp,
            tc.tile_pool(name="dram", bufs=1, space="DRAM") as dp,
        ):
            # persistent sbuf tensors
            wq_sb = pp.tile([P, NC, HH], BF16, tag="wq")
            wk_sb = pp.tile([P, NC, HH], BF16, tag="wk")
            wv_sb = pp.tile([P, NC, HH], BF16, tag="wv")
            wo_sb = pp.tile([P, NC, HH], BF16, tag="wo")
            bq_sb = pp.tile([P, 4], F32, tag="bq")
            bk_sb = pp.tile([P, 4], F32, tag="bk")
            bv_sb = pp.tile([1, HH], F32, tag="bv")
            bo_sb = pp.tile([1, HH], F32, tag="bo")
            tri_sb = pp.tile([P, P], BF16, tag="tri")
            QT = [pp.tile([P, S], BF16, tag=f"qt{p}", name=f"qt{p}")
                  for p in range(NPAIR)]
            KT = [pp.tile([P, S], BF16, tag=f"kt{p}", name=f"kt{p}")
                  for p in range(NPAIR)]
            # V1[s-part, s-tile, head, 128]: every head -> ones in cols 0:64
            # (so PV denominators land on partitions 0:64, where the custom
            # reciprocal op works), V in cols 64:128.
            V1 = pp.tile([P, NT, 8, P], BF16, tag="v1")
            atf = pp.tile([P, NC, S], BF16, tag="atf")
            attnT = [pp.tile([P, S], BF16, tag=f"at{p}", name=f"at{p}")
                     for p in range(NPAIR)]
            bvb = pp.tile([P, HH], F32, tag="bvb") if has_bv else None
            bob = pp.tile([P, HH], F32, tag="bob") if has_bo else None

            own_dram = [dp.tile([P, S], BF16, tag=f"own{p}", name=f"own{p}")
                        for p in range(NPAIR - 1)]
            all_dram = [dp.tile([2, P, S], BF16, tag=f"all{p}", name=f"all{p}")
                        for p in range(NPAIR - 1)]
            # pair 3's exchange is split in column halves so it overlaps the
            # tail of its own attention instead of serializing before phase 3
            own3 = [dp.tile([P, S // 2], BF16, tag=f"own3{h}", name=f"own3{h}")
                    for h in range(2)]
            all3 = [dp.tile([2, P, S // 2], BF16, tag=f"all3{h}",
                            name=f"all3{h}") for h in range(2)]

            # ------------- phase 1: k/v projections -------------
            with (
                tc.tile_pool(name="xt", bufs=1) as xtp,
                tc.tile_pool(name="proj_ps", bufs=8, space="PSUM") as pps,
            ):
                x_sb = [xtp.tile([P, S], BF16, tag=f"xt{c}", name=f"xt{c}")
                        for c in range(NC)]
                # critical path first: wk + kT chunks feed the first matmuls;
                # x chunks split into column quarters for DMA-ring parallelism
                for c in range(NC):
                    nc.sync.dma_start(out=wk_sb[:, c, :], in_=wkT.ap()[:, c, :])
                    for h in range(4):
                        nc.sync.dma_start(
                            out=x_sb[c][:, h * 512:(h + 1) * 512],
                            in_=kT.ap()[:, c, h * 512:(h + 1) * 512])
                nc.sync.dma_start(out=tri_sb[:], in_=trid.ap())
                for b_sb, b_d in ((bq_sb, bqv), (bk_sb, bkv), (bv_sb, bvv),
                                  (bo_sb, bov)):
                    nc.sync.dma_start(out=b_sb[:], in_=b_d.ap())
                for c in range(NC):
                    nc.sync.dma_start(out=wv_sb[:, c, :], in_=wvT.ap()[:, c, :])
                # ones blocks of V1 (written once; V drains only touch V cols)
                nc.vector.memset(V1[:, :, :, 0:64], 1.0)
                if has_bv:
                    nc.gpsimd.partition_broadcast(out=bvb[:], in_=bv_sb[:])
                if has_bo:
                    nc.gpsimd.partition_broadcast(out=bob[:], in_=bo_sb[:])

                # K projection: KT[it] = (Wk x^T)[it*128:(it+1)*128, :]
                for it in range(4):
                    ps = [pps.tile([P, 512], F32, tag="proj", name=f"proj{sb}")
                          for sb in range(4)]
                    for c in range(NC):
                        for sb in range(4):
                            nc.tensor.matmul(
                                ps[sb][:],
                                lhsT=wk_sb[:, c, it * P:(it + 1) * P],
                                rhs=x_sb[c][:, sb * 512:(sb + 1) * 512],
                                start=(c == 0), stop=(c == NC - 1))
                    for sb in range(4):
                        dst = KT[it][:, sb * 512:(sb + 1) * 512]
                        if has_bk:
                            nc.scalar.activation(
                                out=dst, in_=ps[sb][:], func=ID,
                                bias=bk_sb[:, it:it + 1])
                        elif sb % 2 == 0:
                            nc.scalar.copy(out=dst, in_=ps[sb][:])
                        else:
                            nc.vector.tensor_copy(out=dst, in_=ps[sb][:])

                # V projection: V1[:, st, h, vcols] = (x^T)^T Wv per seq tile
                for c in range(NC):
                    for h in range(4):
                        nc.sync.dma_start(
                            out=x_sb[c][:, h * 512:(h + 1) * 512],
                            in_=vT.ap()[:, c, h * 512:(h + 1) * 512])
                for c in range(NC):
                    nc.sync.dma_start(out=wq_sb[:, c, :], in_=wqT.ap()[:, c, :])
                    nc.sync.dma_start(out=wo_sb[:, c, :], in_=woT.ap()[:, c, :])
                for it in range(4):
                    ps = [pps.tile([P, 512], F32, tag="proj", name=f"proj{sb}")
                          for sb in range(4)]
                    for c in range(NC):
                        for sb in range(4):
                            st = it * 4 + sb
                            nc.tensor.matmul(
                                ps[sb][:],
                                lhsT=x_sb[c][:, st * P:(st + 1) * P],
                                rhs=wv_sb[:, c, :],
                                start=(c == 0), stop=(c == NC - 1))
                    for sb in range(4):
                        st = it * 4 + sb
                        pv3 = ps[sb][:].rearrange("p (h d) -> p h d", h=8)
                        if has_bv:
                            bv3 = bvb[:].rearrange("p (h d) -> p h d", h=8)
                            nc.vector.tensor_add(
                                out=V1[:, st, :, 64:128],
                                in0=pv3[:, :, :], in1=bv3[:, :, :])
                        elif sb % 2 == 0:
                            nc.scalar.copy(
                                out=V1[:, st, :, 64:128],
                                in_=pv3[:, :, :])
                        else:
                            nc.vector.tensor_copy(
                                out=V1[:, st, :, 64:128],
                                in_=pv3[:, :, :])

            # ------ phase 2: q projections interleaved with attention ------
            # qproj(pr+1) matmuls are dripped into attention(pr)'s chunk
            # stream (one step per chunk) so ScalarE's exp stream never
            # starves while the PE does projection work.
            with (
                tc.tile_pool(name="xq", bufs=1) as xqp,
                tc.tile_pool(name="pt", bufs=6) as ptp,
                tc.tile_pool(name="rec", bufs=2) as rcp,
                tc.tile_pool(name="lg_ps", bufs=2, space="PSUM") as lgp,
                tc.tile_pool(name="pv_ps", bufs=2, space="PSUM") as pvp,
                tc.tile_pool(name="qp_ps", bufs=2, space="PSUM") as qps,
            ):
                xq = [xqp.tile([P, S], BF16, tag=f"xq{c}", name=f"xq{c}")
                      for c in range(NC)]
                for c in range(NC):
                    for h in range(4):
                        nc.sync.dma_start(
                            out=xq[c][:, h * 512:(h + 1) * 512],
                            in_=qT.ap()[:, c, h * 512:(h + 1) * 512])

                def qproj_steps(pr):
                    # Q projection chunk pr -> QT[pr], one emitted op/step
                    for sb in range(4):
                        qp = qps.tile([P, 512], F32, tag="qp")
                        for c in range(NC):
                            nc.tensor.matmul(
                                qp[:],
                                lhsT=wq_sb[:, c, pr * P:(pr + 1) * P],
                                rhs=xq[c][:, sb * 512:(sb + 1) * 512],
                                start=(c == 0), stop=(c == NC - 1))
                            yield
                        dst = QT[pr][:, sb * 512:(sb + 1) * 512]
                        if has_bq:
                            nc.scalar.activation(
                                out=dst, in_=qp[:], func=ID,
                                bias=bq_sb[:, pr:pr + 1])
                        else:
                            nc.scalar.copy(out=dst, in_=qp[:])
                        yield

                for _ in qproj_steps(0):  # pair 0's projection up front
                    pass

                for pr in range(NPAIR):
                    qstream = qproj_steps(pr + 1) if pr + 1 < NPAIR else None
                    for qb in range(NB):
                        pvA = pvp.tile([P, 512], F32, tag="pv")
                        pvB = pvp.tile([P, 512], F32, tag="pv")
                        nch = qb * 4 + 4
                        q0 = qb * 512

                        def emit_pv(c, pt, off, pvA=pvA, pvB=pvB, pr=pr,
                                    nch=nch):
                            nc.tensor.matmul(
                                pvA[:, off:512],
                                lhsT=V1[:, c, 2 * pr, :],
                                rhs=pt[:, off:512],
                                start=(c == 0), stop=(c == nch - 1))
                            nc.tensor.matmul(
                                pvB[:, off:512],
                                lhsT=V1[:, c, 2 * pr + 1, :],
                                rhs=pt[:, 512 + off:1024],
                                start=(c == 0), stop=(c == nch - 1))

                        pending = []  # (c, pt, off) awaiting PV emission
                        for c in range(nch):
                            jj = c - qb * 4
                            off = 128 * jj if jj > 0 else 0
                            lg = lgp.tile([P, 1024], F32, tag="lg")
                            nc.tensor.matmul(
                                lg[:, off:512],
                                lhsT=KT[pr][0:64, c * P:(c + 1) * P],
                                rhs=QT[pr][0:64, q0 + off:q0 + 512],
                                start=True, stop=True, tile_position=(0, 0))
                            nc.tensor.matmul(
                                lg[:, 512 + off:1024],
                                lhsT=KT[pr][64:128, c * P:(c + 1) * P],
                                rhs=QT[pr][64:128, q0 + off:q0 + 512],
                                start=True, stop=True, tile_position=(64, 0))
                            pt = ptp.tile([P, 1024], BF16, tag="pt")
                            lg3 = lg[:].rearrange("p (h n) -> p h n", h=2)
                            pt3 = pt[:].rearrange("p (h n) -> p h n", h=2)
                            nc.scalar.activation(
                                out=pt3[:, :, off:512],
                                in_=lg3[:, :, off:512],
                                func=mybir.ActivationFunctionType.Exp,
                                scale=0.125)
                            if jj >= 0:  # diagonal 128x128: causal mask
                                nc.vector.tensor_mul(
                                    out=pt[:, off:off + P],
                                    in0=pt[:, off:off + P], in1=tri_sb[:])
                                nc.vector.tensor_mul(
                                    out=pt[:, 512 + off:512 + off + P],
                                    in0=pt[:, 512 + off:512 + off + P],
                                    in1=tri_sb[:])
                            if qstream is not None:
                                next(qstream, None)
                            pending.append((c, pt, off))
                            if len(pending) > 4:
                                emit_pv(*pending.pop(0))
                        for args in pending:
                            emit_pv(*args)
                        # drain: denominators on partitions 0:64 (ones cols),
                        # values on 64:128 for both heads.
                        rec = rcp.tile([P, 1024], F32, tag="rec")
                        nc.vector.reciprocal_approx_fast(
                            rec[0:64, 0:512], pvA[0:64, :])
                        nc.vector.reciprocal_approx_fast(
                            rec[0:64, 512:1024], pvB[0:64, :])
                        nc.vector.tensor_mul(
                            out=attnT[pr][0:64, q0:q0 + 512],
                            in0=pvA[64:128, :], in1=rec[0:64, 0:512])
                        nc.vector.tensor_mul(
                            out=attnT[pr][64:128, q0:q0 + 512],
                            in0=pvB[64:128, :], in1=rec[0:64, 512:1024])
                        # stream this query block out to DRAM immediately so
                        # the pair's exchange isn't gated on one big DMA
                        if pr < NPAIR - 1:
                            nc.sync.dma_start(
                                out=own_dram[pr][:, q0:q0 + 512],
                                in_=attnT[pr][:, q0:q0 + 512])
                        else:
                            nc.sync.dma_start(
                                out=own3[qb // 2][:, (qb % 2) * 512:
                                                  (qb % 2 + 1) * 512],
                                in_=attnT[pr][:, q0:q0 + 512])
                            if qb % 2 == 1:  # half ready: exchange it now
                                hb = qb // 2
                                nc.gpsimd.collective_compute(
                                    "AllGather", mybir.AluOpType.bypass,
                                    replica_groups=REPLICA_GROUPS,
                                    ins=[own3[hb].opt()],
                                    outs=[all3[hb].opt()])
                                for hf in range(2):
                                    nc.sync.dma_start(
                                        out=atf[:, hf * 4 + pr,
                                                hb * 1024:(hb + 1) * 1024],
                                        in_=all3[hb][hf, :, :])
                    if qstream is not None:  # finish any leftover steps
                        for _ in qstream:
                            pass
                    # pair done: exchange with partner core
                    if pr < NPAIR - 1:
                        nc.gpsimd.collective_compute(
                            "AllGather", mybir.AluOpType.bypass,
                            replica_groups=REPLICA_GROUPS,
                            ins=[own_dram[pr].opt()],
                            outs=[all_dram[pr].opt()])
                        for hf in range(2):
                            nc.sync.dma_start(
                                out=atf[:, hf * 4 + pr, 0:1024],
                                in_=all_dram[pr][hf, :, 0:1024])
                            nc.sync.dma_start(
                                out=atf[:, hf * 4 + pr, 1024:2048],
                                in_=all_dram[pr][hf, :, 1024:2048])

            # ---------------- phase 3: output projection ----------------
            with (
                tc.tile_pool(name="ob", bufs=3) as obp,
                tc.tile_pool(name="fc_ps", bufs=8, space="PSUM") as fcp,
            ):
                # pair-3 chunks (ic 3, 7) land last; put them at the end of
                # every accumulation chain so earlier chunks' matmuls can run
                # while the final exchange is still in flight
                ic_order = [0, 1, 2, 4, 5, 6, 3, 7]
                for st in range(NT):
                    fc = fcp.tile([P, 512], F32, tag="fc")
                    for i, ic in enumerate(ic_order):
                        nc.tensor.matmul(
                            fc[:],
                            lhsT=atf[:, ic, st * P:(st + 1) * P],
                            rhs=wo_sb[:, ic, :],
                            start=(i == 0), stop=(i == NC - 1))
                    ob = obp.tile([P, 512], F32, tag="ob")
                    if has_bo:
                        nc.vector.tensor_add(out=ob[:], in0=fc[:], in1=bob[:])
                    else:
                        nc.vector.tensor_copy(out=ob[:], in_=fc[:])
                    nc.sync.dma_start(
                        out=out.ap()[st * P:(st + 1) * P, :], in_=ob[:])

    nc.compile()
    return nc


def _get_nc(flags):
    if flags not in _cache:
        _cache[flags] = _build(*flags)
    return _cache[flags]


def _chunked(xT):
    # [D, cols] -> [128, NC, cols] so each partition's data is contiguous
    cols = xT.shape[1]
    return np.ascontiguousarray(
        xT.reshape(NC, P, cols).transpose(1, 0, 2)).astype(NPBF)


def _prep_inputs(q, k, v, Wq, bq, Wk, bk, Wv, bv, Wo, bo):
    tri = np.triu(np.ones((P, P), np.float32)).astype(NPBF)
    in_maps = []
    for c in range(NCORES):
        b, hf = divmod(c, 2)
        hs = slice(hf * HH, (hf + 1) * HH)
        in_maps.append({
            "qT": _chunked(q[b].T),
            "kT": _chunked(k[b].T),
            "vT": _chunked(v[b].T),
            "wqT": _chunked(Wq[hs].T),
            "wkT": _chunked(Wk[hs].T),
            "wvT": _chunked(Wv[hs].T),
            "woT": _chunked(Wo[hs].T),
            # bq/bk as [128, 4]: partition p, chunk it -> feature it*128+p
            "bq": np.ascontiguousarray(
                np.asarray(bq[hs], np.float32).reshape(4, P).T),
            "bk": np.ascontiguousarray(
                np.asarray(bk[hs], np.float32).reshape(4, P).T),
            "bv": np.asarray(bv[hs], np.float32).reshape(1, HH),
            "bo": np.asarray(bo[hs], np.float32).reshape(1, HH),
            "tri": tri,
        })
    return in_maps


def kernel(q, k, v, mask, Wq, bq, Wk, bk, Wv, bv, Wo, bo, _trace=False):
    q, k, v = (np.asarray(x, np.float32) for x in (q, k, v))
    mask = np.asarray(mask, np.float32)
    exp_mask = np.triu(np.ones((S, S), np.float32), k=1)[None, None]
    assert mask.shape == (1, 1, S, S) and np.array_equal(mask, exp_mask), \
        "kernel specialized for the causal mask produced by setup_inputs()"

    flags = tuple(bool(np.any(np.asarray(x))) for x in (bq, bk, bv, bo))
    nc = _get_nc(flags)
    in_maps = _prep_inputs(q, k, v, Wq, bq, Wk, bk, Wv, bv, Wo, bo)
    res = run_bass_kernel_spmd(nc, in_maps, list(range(NCORES)),
                               trace=_trace)
    if _trace:
        _cache["last_result"] = res
    full = np.empty((B, S, D), np.float32)
    for c in range(NCORES):
        b, hf = divmod(c, 2)
        full[b, :, hf * HH:(hf + 1) * HH] = res.results[c]["out"]
    return full


# revision 18
# speedup vs baseline: 1.2122x; 1.2122x over previous
"""Multi-head attention (B=4, S=2048, D=1024, H=16, causal) on 8 NeuronCores.

Sharding: data-parallel over batch (4) x tensor-parallel over head halves (2).
Core c handles batch c//2 with heads (c%2)*8 .. (c%2)*8+7 and produces output
columns (c%2)*512 .. +512 after an in-pair AllGather of the attention output.

Device pipeline (all bf16 matmuls, fp32 PSUM accumulation):
  phase 1: K and V projections from host-pre-transposed inputs -> KT [i, s]
           per head-pair, V1 [s, (V|ones)] per head with a 64-wide ones block
           so the PV matmul produces softmax denominators on 64 partitions.
  phase 2: per round r: Q projection chunk r (-> QT[r]) immediately followed
           by attention for head pair r, so ScalarE exp work starts ~70us
           earlier and overlaps the remaining projections.  Per head pair /
           512-query block: logits^T tiles [sk=128, sq<=512] via K=64
           row-packed matmul pairs (2 heads concurrent on the PE), exp on
           ScalarE (scale=1/8 fused), causal handled by restricting computed
           ranges + a triangular-mask multiply on the diagonal 128x128 tile,
           PV matmul with full M=128 (64 value cols + 64 ones cols).
           Softmax division uses vector.reciprocal_approx_fast (~5x faster
           than the iterative DVE reciprocal, ~18 correct bits).
  phase 3: pair-wise AllGather of attnT (issued per-pair, overlapped), then
           the output projection for this core's 512 output columns.

Biases are folded in only when nonzero (they are all zero for this model's
inputs): bq/bk via per-partition activation bias on the projection drains,
bv/bo via partition-broadcast tiles added on the V1/output drains.  No PE
cycles are ever spent on biases.
"""

import numpy as np
import ml_dtypes

import concourse.bass as bass
import concourse.mybir as mybir
import concourse.tile as tile
from concourse import bacc
from concourse.bass_utils import run_bass_kernel_spmd

B, S, D, H = 4, 2048, 1024, 16
HD = D // H  # 64
NCORES = 8
HH = D // 2  # 512 = head-half width (8 heads x 64) = output col split
BF16 = mybir.dt.bfloat16
F32 = mybir.dt.float32
NPBF = ml_dtypes.bfloat16

P = 128          # partitions
NB = S // 512    # 4 query/seq blocks of 512
NT = S // P      # 16 seq tiles of 128
NC = D // P      # 8 contraction chunks of 128
NPAIR = 4        # head pairs per core

REPLICA_GROUPS = [[0, 1], [2, 3], [4, 5], [6, 7]]

_cache = {}


def _build(has_bq, has_bk, has_bv, has_bo):
    nc = bacc.Bacc("TRN2", target_bir_lowering=False, debug=False,
                   num_devices=NCORES)

    # ---- dram I/O ----
    qT = nc.dram_tensor("qT", [P, NC, S], BF16, kind="ExternalInput")
    kT = nc.dram_tensor("kT", [P, NC, S], BF16, kind="ExternalInput")
    vT = nc.dram_tensor("vT", [P, NC, S], BF16, kind="ExternalInput")
    wqT = nc.dram_tensor("wqT", [P, NC, HH], BF16, kind="ExternalInput")
    wkT = nc.dram_tensor("wkT", [P, NC, HH], BF16, kind="ExternalInput")
    wvT = nc.dram_tensor("wvT", [P, NC, HH], BF16, kind="ExternalInput")
    woT = nc.dram_tensor("woT", [P, NC, HH], BF16, kind="ExternalInput")
    bqv = nc.dram_tensor("bq", [P, 4], F32, kind="ExternalInput")
    bkv = nc.dram_tensor("bk", [P, 4], F32, kind="ExternalInput")
    bvv = nc.dram_tensor("bv", [1, HH], F32, kind="ExternalInput")
    bov = nc.dram_tensor("bo", [1, HH], F32, kind="ExternalInput")
    trid = nc.dram_tensor("tri", [P, P], BF16, kind="ExternalInput")
    out = nc.dram_tensor("out", [S, HH], F32, kind="ExternalOutput")

    ID = mybir.ActivationFunctionType.Identity

    with tile.TileContext(nc) as tc:
        with (
            tc.tile_pool(name="persist", bufs=1) as pp,
            tc.tile_pool(name="dram", bufs=1, space="DRAM") as dp,
        ):
            # persistent sbuf tensors
            wq_sb = pp.tile([P, NC, HH], BF16, tag="wq")
            wk_sb = pp.tile([P, NC, HH], BF16, tag="wk")
            wv_sb = pp.tile([P, NC, HH], BF16, tag="wv")
            wo_sb = pp.tile([P, NC, HH], BF16, tag="wo")
            bq_sb = pp.tile([P, 4], F32, tag="bq")
            bk_sb = pp.tile([P, 4], F32, tag="bk")
            bv_sb = pp.tile([1, HH], F32, tag="bv")
            bo_sb = pp.tile([1, HH], F32, tag="bo")
            tri_sb = pp.tile([P, P], BF16, tag="tri")
            QT = [pp.tile([P, S], BF16, tag=f"qt{p}", name=f"qt{p}")
                  for p in range(NPAIR)]
            KT = [pp.tile([P, S], BF16, tag=f"kt{p}", name=f"kt{p}")
                  for p in range(NPAIR)]
            # V1[s-part, s-tile, head, 128]: every head -> ones in cols 0:64
            # (so PV denominators land on partitions 0:64, where the custom
            # reciprocal op works), V in cols 64:128.
            V1 = pp.tile([P, NT, 8, P], BF16, tag="v1")
            atf = pp.tile([P, NC, S], BF16, tag="atf")
            attnT = [pp.tile([P, S], BF16, tag=f"at{p}", name=f"at{p}")
                     for p in range(NPAIR)]
            bvb = pp.tile([P, HH], F32, tag="bvb") if has_bv else None
            bob = pp.tile([P, HH], F32, tag="bob") if has_bo else None

            own_dram = [dp.tile([P, S], BF16, tag=f"own{p}", name=f"own{p}")
                        for p in range(NPAIR - 1)]
            all_dram = [dp.tile([2, P, S], BF16, tag=f"all{p}", name=f"all{p}")
                        for p in range(NPAIR - 1)]
            # pair 3's exchange is split in column halves so it overlaps the
            # tail of its own attention instead of serializing before phase 3
            own3 = [dp.tile([P, S // 2], BF16, tag=f"own3{h}", name=f"own3{h}")
                    for h in range(2)]
            all3 = [dp.tile([2, P, S // 2], BF16, tag=f"all3{h}",
                            name=f"all3{h}") for h in range(2)]

            # ------------- phase 1: k/v projections -------------
            with (
                tc.tile_pool(name="xt", bufs=1) as xtp,
                tc.tile_pool(name="proj_ps", bufs=8, space="PSUM") as pps,
            ):
                x_sb = [xtp.tile([P, S], BF16, tag=f"xt{c}", name=f"xt{c}")
                        for c in range(NC)]
                # critical path first: wk + kT chunks feed the first matmuls
                for c in range(NC):
                    nc.sync.dma_start(out=wk_sb[:, c, :], in_=wkT.ap()[:, c, :])
                    nc.sync.dma_start(out=x_sb[c][:], in_=kT.ap()[:, c, :])
                nc.sync.dma_start(out=tri_sb[:], in_=trid.ap())
                for b_sb, b_d in ((bq_sb, bqv), (bk_sb, bkv), (bv_sb, bvv),
                                  (bo_sb, bov)):
                    nc.sync.dma_start(out=b_sb[:], in_=b_d.ap())
                for c in range(NC):
                    nc.sync.dma_start(out=wv_sb[:, c, :], in_=wvT.ap()[:, c, :])
                # ones blocks of V1 (written once; V drains only touch V cols)
                nc.vector.memset(V1[:, :, :, 0:64], 1.0)
                if has_bv:
                    nc.gpsimd.partition_broadcast(out=bvb[:], in_=bv_sb[:])
                if has_bo:
                    nc.gpsimd.partition_broadcast(out=bob[:], in_=bo_sb[:])

                # K projection: KT[it] = (Wk x^T)[it*128:(it+1)*128, :]
                for it in range(4):
                    ps = [pps.tile([P, 512], F32, tag="proj", name=f"proj{sb}")
                          for sb in range(4)]
                    for c in range(NC):
                        for sb in range(4):
                            nc.tensor.matmul(
                                ps[sb][:],
                                lhsT=wk_sb[:, c, it * P:(it + 1) * P],
                                rhs=x_sb[c][:, sb * 512:(sb + 1) * 512],
                                start=(c == 0), stop=(c == NC - 1))
                    for sb in range(4):
                        dst = KT[it][:, sb * 512:(sb + 1) * 512]
                        if has_bk:
                            nc.scalar.activation(
                                out=dst, in_=ps[sb][:], func=ID,
                                bias=bk_sb[:, it:it + 1])
                        else:
                            nc.scalar.copy(out=dst, in_=ps[sb][:])

                # V projection: V1[:, st, h, vcols] = (x^T)^T Wv per seq tile
                for c in range(NC):
                    nc.sync.dma_start(out=x_sb[c][:], in_=vT.ap()[:, c, :])
                for c in range(NC):
                    nc.sync.dma_start(out=wq_sb[:, c, :], in_=wqT.ap()[:, c, :])
                    nc.sync.dma_start(out=wo_sb[:, c, :], in_=woT.ap()[:, c, :])
                for it in range(4):
                    ps = [pps.tile([P, 512], F32, tag="proj", name=f"proj{sb}")
                          for sb in range(4)]
                    for c in range(NC):
                        for sb in range(4):
                            st = it * 4 + sb
                            nc.tensor.matmul(
                                ps[sb][:],
                                lhsT=x_sb[c][:, st * P:(st + 1) * P],
                                rhs=wv_sb[:, c, :],
                                start=(c == 0), stop=(c == NC - 1))
                    for sb in range(4):
                        st = it * 4 + sb
                        pv3 = ps[sb][:].rearrange("p (h d) -> p h d", h=8)
                        if has_bv:
                            bv3 = bvb[:].rearrange("p (h d) -> p h d", h=8)
                            nc.vector.tensor_add(
                                out=V1[:, st, :, 64:128],
                                in0=pv3[:, :, :], in1=bv3[:, :, :])
                        else:
                            nc.scalar.copy(
                                out=V1[:, st, :, 64:128],
                                in_=pv3[:, :, :])

            # ------ phase 2: q projections interleaved with attention ------
            # qproj(pr+1) matmuls are dripped into attention(pr)'s chunk
            # stream (one step per chunk) so ScalarE's exp stream never
            # starves while the PE does projection work.
            with (
                tc.tile_pool(name="xq", bufs=1) as xqp,
                tc.tile_pool(name="pt", bufs=6) as ptp,
                tc.tile_pool(name="rec", bufs=2) as rcp,
                tc.tile_pool(name="lg_ps", bufs=2, space="PSUM") as lgp,
                tc.tile_pool(name="pv_ps", bufs=2, space="PSUM") as pvp,
                tc.tile_pool(name="qp_ps", bufs=2, space="PSUM") as qps,
            ):
                xq = [xqp.tile([P, S], BF16, tag=f"xq{c}", name=f"xq{c}")
                      for c in range(NC)]
                for c in range(NC):
                    nc.sync.dma_start(out=xq[c][:], in_=qT.ap()[:, c, :])

                def qproj_steps(pr):
                    # Q projection chunk pr -> QT[pr], one emitted op/step
                    for sb in range(4):
                        qp = qps.tile([P, 512], F32, tag="qp")
                        for c in range(NC):
                            nc.tensor.matmul(
                                qp[:],
                                lhsT=wq_sb[:, c, pr * P:(pr + 1) * P],
                                rhs=xq[c][:, sb * 512:(sb + 1) * 512],
                                start=(c == 0), stop=(c == NC - 1))
                            yield
                        dst = QT[pr][:, sb * 512:(sb + 1) * 512]
                        if has_bq:
                            nc.scalar.activation(
                                out=dst, in_=qp[:], func=ID,
                                bias=bq_sb[:, pr:pr + 1])
                        else:
                            nc.scalar.copy(out=dst, in_=qp[:])
                        yield

                for _ in qproj_steps(0):  # pair 0's projection up front
                    pass

                for pr in range(NPAIR):
                    qstream = qproj_steps(pr + 1) if pr + 1 < NPAIR else None
                    for qb in range(NB):
                        pvA = pvp.tile([P, 512], F32, tag="pv")
                        pvB = pvp.tile([P, 512], F32, tag="pv")
                        nch = qb * 4 + 4
                        q0 = qb * 512

                        def emit_pv(c, pt, off, pvA=pvA, pvB=pvB, pr=pr,
                                    nch=nch):
                            nc.tensor.matmul(
                                pvA[:, off:512],
                                lhsT=V1[:, c, 2 * pr, :],
                                rhs=pt[:, off:512],
                                start=(c == 0), stop=(c == nch - 1))
                            nc.tensor.matmul(
                                pvB[:, off:512],
                                lhsT=V1[:, c, 2 * pr + 1, :],
                                rhs=pt[:, 512 + off:1024],
                                start=(c == 0), stop=(c == nch - 1))

                        pending = []  # (c, pt, off) awaiting PV emission
                        for c in range(nch):
                            jj = c - qb * 4
                            off = 128 * jj if jj > 0 else 0
                            lg = lgp.tile([P, 1024], F32, tag="lg")
                            nc.tensor.matmul(
                                lg[:, off:512],
                                lhsT=KT[pr][0:64, c * P:(c + 1) * P],
                                rhs=QT[pr][0:64, q0 + off:q0 + 512],
                                start=True, stop=True, tile_position=(0, 0))
                            nc.tensor.matmul(
                                lg[:, 512 + off:1024],
                                lhsT=KT[pr][64:128, c * P:(c + 1) * P],
                                rhs=QT[pr][64:128, q0 + off:q0 + 512],
                                start=True, stop=True, tile_position=(64, 0))
                            pt = ptp.tile([P, 1024], BF16, tag="pt")
                            lg3 = lg[:].rearrange("p (h n) -> p h n", h=2)
                            pt3 = pt[:].rearrange("p (h n) -> p h n", h=2)
                            nc.scalar.activation(
                                out=pt3[:, :, off:512],
                                in_=lg3[:, :, off:512],
                                func=mybir.ActivationFunctionType.Exp,
                                scale=0.125)
                            if jj >= 0:  # diagonal 128x128: causal mask
                                nc.vector.tensor_mul(
                                    out=pt[:, off:off + P],
                                    in0=pt[:, off:off + P], in1=tri_sb[:])
                                nc.vector.tensor_mul(
                                    out=pt[:, 512 + off:512 + off + P],
                                    in0=pt[:, 512 + off:512 + off + P],
                                    in1=tri_sb[:])
                            if qstream is not None:
                                next(qstream, None)
                            pending.append((c, pt, off))
                            if len(pending) > 4:
                                emit_pv(*pending.pop(0))
                        for args in pending:
                            emit_pv(*args)
                        # drain: denominators on partitions 0:64 (ones cols),
                        # values on 64:128 for both heads.
                        rec = rcp.tile([P, 1024], F32, tag="rec")
                        nc.vector.reciprocal_approx_fast(
                            rec[0:64, 0:512], pvA[0:64, :])
                        nc.vector.reciprocal_approx_fast(
                            rec[0:64, 512:1024], pvB[0:64, :])
                        nc.vector.tensor_mul(
                            out=attnT[pr][0:64, q0:q0 + 512],
                            in0=pvA[64:128, :], in1=rec[0:64, 0:512])
                        nc.vector.tensor_mul(
                            out=attnT[pr][64:128, q0:q0 + 512],
                            in0=pvB[64:128, :], in1=rec[0:64, 512:1024])
                        # stream this query block out to DRAM immediately so
                        # the pair's exchange isn't gated on one big DMA
                        if pr < NPAIR - 1:
                            nc.sync.dma_start(
                                out=own_dram[pr][:, q0:q0 + 512],
                                in_=attnT[pr][:, q0:q0 + 512])
                        else:
                            nc.sync.dma_start(
                                out=own3[qb // 2][:, (qb % 2) * 512:
                                                  (qb % 2 + 1) * 512],
                                in_=attnT[pr][:, q0:q0 + 512])
                            if qb % 2 == 1:  # half ready: exchange it now
                                hb = qb // 2
                                nc.gpsimd.collective_compute(
                                    "AllGather", mybir.AluOpType.bypass,
                                    replica_groups=REPLICA_GROUPS,
                                    ins=[own3[hb].opt()],
                                    outs=[all3[hb].opt()])
                                for hf in range(2):
                                    nc.sync.dma_start(
                                        out=atf[:, hf * 4 + pr,
                                                hb * 1024:(hb + 1) * 1024],
                                        in_=all3[hb][hf, :, :])
                    if qstream is not None:  # finish any leftover steps
                        for _ in qstream:
                            pass
                    # pair done: exchange with partner core
                    if pr < NPAIR - 1:
                        nc.gpsimd.collective_compute(
                            "AllGather", mybir.AluOpType.bypass,
                            replica_groups=REPLICA_GROUPS,
                            ins=[own_dram[pr].opt()],
                            outs=[all_dram[pr].opt()])
                        for hf in range(2):
                            nc.sync.dma_start(
                                out=atf[:, hf * 4 + pr, 0:1024],
                                in_=all_dram[pr][hf, :, 0:1024])
                            nc.sync.dma_start(
                                out=atf[:, hf * 4 + pr, 1024:2048],
                                in_=all_dram[pr][hf, :, 1024:2048])

            # ---------------- phase 3: output projection ----------------
            with (
                tc.tile_pool(name="ob", bufs=3) as obp,
                tc.tile_pool(name="fc_ps", bufs=8, space="PSUM") as fcp,
            ):
                # pair-3 chunks (ic 3, 7) land last; put them at the end of
                # every accumulation chain so earlier chunks' matmuls can run
                # while the final exchange is still in flight
                ic_order = [0, 1, 2, 4, 5, 6, 3, 7]
                for st in range(NT):
                    fc = fcp.tile([P, 512], F32, tag="fc")
                    for i, ic in enumerate(ic_order):
                        nc.tensor.matmul(
                            fc[:],
                            lhsT=atf[:, ic, st * P:(st + 1) * P],
                            rhs=wo_sb[:, ic, :],
                            start=(i == 0), stop=(i == NC - 1))
                    ob = obp.tile([P, 512], F32, tag="ob")
                    if has_bo:
                        nc.vector.tensor_add(out=ob[:], in0=fc[:], in1=bob[:])
                    else:
                        nc.vector.tensor_copy(out=ob[:], in_=fc[:])
                    nc.sync.dma_start(
                        out=out.ap()[st * P:(st + 1) * P, :], in_=ob[:])

    nc.compile()
    return nc


def _get_nc(flags):
    if flags not in _cache:
        _cache[flags] = _build(*flags)
    return _cache[flags]


def _chunked(xT):
    # [D, cols] -> [128, NC, cols] so each partition's data is contiguous
    cols = xT.shape[1]
    return np.ascontiguousarray(
        xT.reshape(NC, P, cols).transpose(1, 0, 2)).astype(NPBF)


def _prep_inputs(q, k, v, Wq, bq, Wk, bk, Wv, bv, Wo, bo):
    tri = np.triu(np.ones((P, P), np.float32)).astype(NPBF)
    in_maps = []
    for c in range(NCORES):
        b, hf = divmod(c, 2)
        hs = slice(hf * HH, (hf + 1) * HH)
        in_maps.append({
            "qT": _chunked(q[b].T),
            "kT": _chunked(k[b].T),
            "vT": _chunked(v[b].T),
            "wqT": _chunked(Wq[hs].T),
            "wkT": _chunked(Wk[hs].T),
            "wvT": _chunked(Wv[hs].T),
            "woT": _chunked(Wo[hs].T),
            # bq/bk as [128, 4]: partition p, chunk it -> feature it*128+p
            "bq": np.ascontiguousarray(
                np.asarray(bq[hs], np.float32).reshape(4, P).T),
            "bk": np.ascontiguousarray(
                np.asarray(bk[hs], np.float32).reshape(4, P).T),
            "bv": np.asarray(bv[hs], np.float32).reshape(1, HH),
            "bo": np.asarray(bo[hs], np.float32).reshape(1, HH),
            "tri": tri,
        })
    return in_maps


def kernel(q, k, v, mask, Wq, bq, Wk, bk, Wv, bv, Wo, bo, _trace=False):
    q, k, v = (np.asarray(x, np.float32) for x in (q, k, v))
    mask = np.asarray(mask, np.float32)
    exp_mask = np.triu(np.ones((S, S), np.float32), k=1)[None, None]
    assert mask.shape == (1, 1, S, S) and np.array_equal(mask, exp_mask), \
        "kernel specialized for the causal mask produced by setup_inputs()"

    flags = tuple(bool(np.any(np.asarray(x))) for x in (bq, bk, bv, bo))
    nc = _get_nc(flags)
    in_maps = _prep_inputs(q, k, v, Wq, bq, Wk, bk, Wv, bv, Wo, bo)
    res = run_bass_kernel_spmd(nc, in_maps, list(range(NCORES)),
                               trace=_trace)
    if _trace:
        _cache["last_result"] = res
    full = np.empty((B, S, D), np.float32)
    for c in range(NCORES):
        b, hf = divmod(c, 2)
        full[b, :, hf * HH:(hf + 1) * HH] = res.results[c]["out"]
    return full


# revision 26
# speedup vs baseline: 1.2222x; 1.0083x over previous
"""Multi-head attention (B=4, S=2048, D=1024, H=16, causal) on 8 NeuronCores.

Sharding: data-parallel over batch (4) x tensor-parallel over head halves (2).
Core c handles batch c//2 with heads (c%2)*8 .. (c%2)*8+7 and produces output
columns (c%2)*512 .. +512 after an in-pair AllGather of the attention output.

Device pipeline (all bf16 matmuls, fp32 PSUM accumulation):
  phase 1: K and V projections from host-pre-transposed inputs -> KT [i, s]
           per head-pair, V1 [s, (V|ones)] per head with a 64-wide ones block
           so the PV matmul produces softmax denominators on 64 partitions.
  phase 2: per round r: Q projection chunk r (-> QT[r]) immediately followed
           by attention for head pair r, so ScalarE exp work starts ~70us
           earlier and overlaps the remaining projections.  Per head pair /
           512-query block: logits^T tiles [sk=128, sq<=512] via K=64
           row-packed matmul pairs (2 heads concurrent on the PE), exp on
           ScalarE (scale=1/8 fused), causal handled by restricting computed
           ranges + a triangular-mask multiply on the diagonal 128x128 tile,
           PV matmul with full M=128 (64 value cols + 64 ones cols).
           Softmax division uses vector.reciprocal_approx_fast (~5x faster
           than the iterative DVE reciprocal, ~18 correct bits).
  phase 3: pair-wise AllGather of attnT (issued per-pair, overlapped), then
           the output projection for this core's 512 output columns.

Biases are folded in only when nonzero (they are all zero for this model's
inputs): bq/bk via per-partition activation bias on the projection drains,
bv/bo via partition-broadcast tiles added on the V1/output drains.  No PE
cycles are ever spent on biases.
"""

import numpy as np
import ml_dtypes

import concourse.bass as bass
import concourse.mybir as mybir
import concourse.tile as tile
from concourse import bacc
from concourse.bass_utils import run_bass_kernel_spmd

B, S, D, H = 4, 2048, 1024, 16
HD = D // H  # 64
NCORES = 8
HH = D // 2  # 512 = head-half width (8 heads x 64) = output col split
BF16 = mybir.dt.bfloat16
F32 = mybir.dt.float32
NPBF = ml_dtypes.bfloat16

P = 128          # partitions
NB = S // 512    # 4 query/seq blocks of 512
NT = S // P      # 16 seq tiles of 128
NC = D // P      # 8 contraction chunks of 128
NPAIR = 4        # head pairs per core

REPLICA_GROUPS = [[0, 1], [2, 3], [4, 5], [6, 7]]

_cache = {}


def _build(has_bq, has_bk, has_bv, has_bo):
    nc = bacc.Bacc("TRN2", target_bir_lowering=False, debug=False,
                   num_devices=NCORES)

    # ---- dram I/O ----
    qT = nc.dram_tensor("qT", [P, NC, S], BF16, kind="ExternalInput")
    kT = nc.dram_tensor("kT", [P, NC, S], BF16, kind="ExternalInput")
    vT = nc.dram_tensor("vT", [P, NC, S], BF16, kind="ExternalInput")
    wqT = nc.dram_tensor("wqT", [P, NC, HH], BF16, kind="ExternalInput")
    wkT = nc.dram_tensor("wkT", [P, NC, HH], BF16, kind="ExternalInput")
    wvT = nc.dram_tensor("wvT", [P, NC, HH], BF16, kind="ExternalInput")
    woT = nc.dram_tensor("woT", [P, NC, HH], BF16, kind="ExternalInput")
    bqv = nc.dram_tensor("bq", [P, 4], F32, kind="ExternalInput")
    bkv = nc.dram_tensor("bk", [P, 4], F32, kind="ExternalInput")
    bvv = nc.dram_tensor("bv", [1, HH], F32, kind="ExternalInput")
    bov = nc.dram_tensor("bo", [1, HH], F32, kind="ExternalInput")
    trid = nc.dram_tensor("tri", [P, P], BF16, kind="ExternalInput")
    out = nc.dram_tensor("out", [S, HH], F32, kind="ExternalOutput")

    ID = mybir.ActivationFunctionType.Identity

    with tile.TileContext(nc) as tc:
        with (
            tc.tile_pool(name="persist", bufs=1) as pp,
            tc.tile_pool(name="dram", bufs=1, space="DRAM") as dp,
        ):
            # persistent sbuf tensors
            wq_sb = pp.tile([P, NC, HH], BF16, tag="wq")
            wk_sb = pp.tile([P, NC, HH], BF16, tag="wk")
            wv_sb = pp.tile([P, NC, HH], BF16, tag="wv")
            wo_sb = pp.tile([P, NC, HH], BF16, tag="wo")
            bq_sb = pp.tile([P, 4], F32, tag="bq")
            bk_sb = pp.tile([P, 4], F32, tag="bk")
            bv_sb = pp.tile([1, HH], F32, tag="bv")
            bo_sb = pp.tile([1, HH], F32, tag="bo")
            tri_sb = pp.tile([P, P], BF16, tag="tri")
            QT = [pp.tile([P, S], BF16, tag=f"qt{p}", name=f"qt{p}")
                  for p in range(NPAIR)]
            KT = [pp.tile([P, S], BF16, tag=f"kt{p}", name=f"kt{p}")
                  for p in range(NPAIR)]
            # V1[s-part, s-tile, head, 128]: every head -> ones in cols 0:64
            # (so PV denominators land on partitions 0:64, where the custom
            # reciprocal op works), V in cols 64:128.
            V1 = pp.tile([P, NT, 8, P], BF16, tag="v1")
            attnT = [pp.tile([P, S], BF16, tag=f"at{p}", name=f"at{p}")
                     for p in range(NPAIR)]
            bvb = pp.tile([P, HH], F32, tag="bvb") if has_bv else None
            bob = pp.tile([P, HH], F32, tag="bob") if has_bo else None

            own_dram = [dp.tile([P, S], BF16, tag=f"own{p}", name=f"own{p}")
                        for p in range(NPAIR - 1)]
            all_dram = [dp.tile([2, P, S], BF16, tag=f"all{p}", name=f"all{p}")
                        for p in range(NPAIR - 1)]
            # pair 3's exchange is split in column halves so it overlaps the
            # tail of its own attention instead of serializing before phase 3
            own3 = [dp.tile([P, S // 2], BF16, tag=f"own3{h}", name=f"own3{h}")
                    for h in range(2)]
            all3 = [dp.tile([2, P, S // 2], BF16, tag=f"all3{h}",
                            name=f"all3{h}") for h in range(2)]

            # ------------- phase 1: k/v projections -------------
            with (
                tc.tile_pool(name="xt", bufs=1) as xtp,
                tc.tile_pool(name="proj_ps", bufs=8, space="PSUM") as pps,
            ):
                x_sb = [xtp.tile([P, S], BF16, tag=f"xt{c}", name=f"xt{c}")
                        for c in range(NC)]
                xv_sb = [xtp.tile([P, S], BF16, tag=f"xv{c}", name=f"xv{c}")
                         for c in range(NC)]
                # critical path first: wk + kT chunks feed the first matmuls
                for c in range(NC):
                    nc.sync.dma_start(out=wk_sb[:, c, :], in_=wkT.ap()[:, c, :])
                    nc.sync.dma_start(out=x_sb[c][:], in_=kT.ap()[:, c, :])
                nc.sync.dma_start(out=tri_sb[:], in_=trid.ap())
                for b_sb, b_d in ((bq_sb, bqv), (bk_sb, bkv), (bv_sb, bvv),
                                  (bo_sb, bov)):
                    nc.sync.dma_start(out=b_sb[:], in_=b_d.ap())
                for c in range(NC):
                    nc.sync.dma_start(out=wv_sb[:, c, :], in_=wvT.ap()[:, c, :])
                    nc.sync.dma_start(out=xv_sb[c][:], in_=vT.ap()[:, c, :])
                # ones blocks of V1 (written once; V drains only touch V cols)
                nc.vector.memset(V1[:, :, :, 0:64], 1.0)
                if has_bv:
                    nc.gpsimd.partition_broadcast(out=bvb[:], in_=bv_sb[:])
                if has_bo:
                    nc.gpsimd.partition_broadcast(out=bob[:], in_=bo_sb[:])

                # K projection: KT[it] = (Wk x^T)[it*128:(it+1)*128, :]
                for it in range(4):
                    ps = [pps.tile([P, 512], F32, tag="proj", name=f"proj{sb}")
                          for sb in range(4)]
                    for c in range(NC):
                        for sb in range(4):
                            nc.tensor.matmul(
                                ps[sb][:],
                                lhsT=wk_sb[:, c, it * P:(it + 1) * P],
                                rhs=x_sb[c][:, sb * 512:(sb + 1) * 512],
                                start=(c == 0), stop=(c == NC - 1))
                    for sb in range(4):
                        dst = KT[it][:, sb * 512:(sb + 1) * 512]
                        if has_bk:
                            nc.scalar.activation(
                                out=dst, in_=ps[sb][:], func=ID,
                                bias=bk_sb[:, it:it + 1])
                        else:
                            nc.scalar.copy(out=dst, in_=ps[sb][:])

                # V projection: V1[:, st, h, vcols] = (x^T)^T Wv per seq tile
                for c in range(NC):
                    nc.sync.dma_start(out=wq_sb[:, c, :], in_=wqT.ap()[:, c, :])
                    nc.sync.dma_start(out=wo_sb[:, c, :], in_=woT.ap()[:, c, :])
                for it in range(4):
                    ps = [pps.tile([P, 512], F32, tag="proj", name=f"proj{sb}")
                          for sb in range(4)]
                    for c in range(NC):
                        for sb in range(4):
                            st = it * 4 + sb
                            nc.tensor.matmul(
                                ps[sb][:],
                                lhsT=xv_sb[c][:, st * P:(st + 1) * P],
                                rhs=wv_sb[:, c, :],
                                start=(c == 0), stop=(c == NC - 1))
                    for sb in range(4):
                        st = it * 4 + sb
                        pv3 = ps[sb][:].rearrange("p (h d) -> p h d", h=8)
                        if has_bv:
                            bv3 = bvb[:].rearrange("p (h d) -> p h d", h=8)
                            nc.vector.tensor_add(
                                out=V1[:, st, :, 64:128],
                                in0=pv3[:, :, :], in1=bv3[:, :, :])
                        else:
                            nc.scalar.copy(
                                out=V1[:, st, :, 64:128],
                                in_=pv3[:, :, :])

            # atf lives only in phases 2+3; freeing its 32KB/partition during
            # phase 1 makes room for the separate K/V input tile sets
            afp = tc.alloc_tile_pool(name="atfp", bufs=1)
            atf = afp.tile([P, NC, S], BF16, tag="atf")

            # ------ phase 2: q projections interleaved with attention ------
            # qproj(pr+1) matmuls are dripped into attention(pr)'s chunk
            # stream (one step per chunk) so ScalarE's exp stream never
            # starves while the PE does projection work.
            with (
                tc.tile_pool(name="xq", bufs=1) as xqp,
                tc.tile_pool(name="pt", bufs=6) as ptp,
                tc.tile_pool(name="rec", bufs=2) as rcp,
                tc.tile_pool(name="lg_ps", bufs=2, space="PSUM") as lgp,
                tc.tile_pool(name="pv_ps", bufs=2, space="PSUM") as pvp,
                tc.tile_pool(name="qp_ps", bufs=2, space="PSUM") as qps,
            ):
                xq = [xqp.tile([P, S], BF16, tag=f"xq{c}", name=f"xq{c}")
                      for c in range(NC)]
                for c in range(NC):
                    nc.sync.dma_start(out=xq[c][:], in_=qT.ap()[:, c, :])

                def qproj_steps(pr):
                    # Q projection chunk pr -> QT[pr], one emitted op/step
                    for sb in range(4):
                        qp = qps.tile([P, 512], F32, tag="qp")
                        for c in range(NC):
                            nc.tensor.matmul(
                                qp[:],
                                lhsT=wq_sb[:, c, pr * P:(pr + 1) * P],
                                rhs=xq[c][:, sb * 512:(sb + 1) * 512],
                                start=(c == 0), stop=(c == NC - 1))
                            yield
                        dst = QT[pr][:, sb * 512:(sb + 1) * 512]
                        if has_bq:
                            nc.scalar.activation(
                                out=dst, in_=qp[:], func=ID,
                                bias=bq_sb[:, pr:pr + 1])
                        else:
                            nc.scalar.copy(out=dst, in_=qp[:])
                        yield

                for _ in qproj_steps(0):  # pair 0's projection up front
                    pass

                for pr in range(NPAIR):
                    qstream = qproj_steps(pr + 1) if pr + 1 < NPAIR else None
                    for qb in range(NB):
                        pvA = pvp.tile([P, 512], F32, tag="pv")
                        pvB = pvp.tile([P, 512], F32, tag="pv")
                        nch = qb * 4 + 4
                        q0 = qb * 512

                        def emit_pv(c, pt, off, pvA=pvA, pvB=pvB, pr=pr,
                                    nch=nch):
                            nc.tensor.matmul(
                                pvA[:, off:512],
                                lhsT=V1[:, c, 2 * pr, :],
                                rhs=pt[:, off:512],
                                start=(c == 0), stop=(c == nch - 1))
                            nc.tensor.matmul(
                                pvB[:, off:512],
                                lhsT=V1[:, c, 2 * pr + 1, :],
                                rhs=pt[:, 512 + off:1024],
                                start=(c == 0), stop=(c == nch - 1))

                        pending = []  # (c, pt, off) awaiting PV emission
                        for c in range(nch):
                            jj = c - qb * 4
                            off = 128 * jj if jj > 0 else 0
                            lg = lgp.tile([P, 1024], F32, tag="lg")
                            nc.tensor.matmul(
                                lg[:, off:512],
                                lhsT=KT[pr][0:64, c * P:(c + 1) * P],
                                rhs=QT[pr][0:64, q0 + off:q0 + 512],
                                start=True, stop=True, tile_position=(0, 0))
                            nc.tensor.matmul(
                                lg[:, 512 + off:1024],
                                lhsT=KT[pr][64:128, c * P:(c + 1) * P],
                                rhs=QT[pr][64:128, q0 + off:q0 + 512],
                                start=True, stop=True, tile_position=(64, 0))
                            pt = ptp.tile([P, 1024], BF16, tag="pt")
                            lg3 = lg[:].rearrange("p (h n) -> p h n", h=2)
                            pt3 = pt[:].rearrange("p (h n) -> p h n", h=2)
                            nc.scalar.activation(
                                out=pt3[:, :, off:512],
                                in_=lg3[:, :, off:512],
                                func=mybir.ActivationFunctionType.Exp,
                                scale=0.125)
                            if jj >= 0:  # diagonal 128x128: causal mask
                                nc.vector.tensor_mul(
                                    out=pt[:, off:off + P],
                                    in0=pt[:, off:off + P], in1=tri_sb[:])
                                nc.vector.tensor_mul(
                                    out=pt[:, 512 + off:512 + off + P],
                                    in0=pt[:, 512 + off:512 + off + P],
                                    in1=tri_sb[:])
                            if qstream is not None:
                                next(qstream, None)
                            pending.append((c, pt, off))
                            if len(pending) > 4:
                                emit_pv(*pending.pop(0))
                        for args in pending:
                            emit_pv(*args)
                        # drain: denominators on partitions 0:64 (ones cols),
                        # values on 64:128 for both heads.
                        rec = rcp.tile([P, 1024], F32, tag="rec")
                        nc.vector.reciprocal_approx_fast(
                            rec[0:64, 0:512], pvA[0:64, :])
                        nc.vector.reciprocal_approx_fast(
                            rec[0:64, 512:1024], pvB[0:64, :])
                        nc.vector.tensor_mul(
                            out=attnT[pr][0:64, q0:q0 + 512],
                            in0=pvA[64:128, :], in1=rec[0:64, 0:512])
                        nc.vector.tensor_mul(
                            out=attnT[pr][64:128, q0:q0 + 512],
                            in0=pvB[64:128, :], in1=rec[0:64, 512:1024])
                        # stream this query block out to DRAM immediately so
                        # the pair's exchange isn't gated on one big DMA
                        if pr < NPAIR - 1:
                            nc.sync.dma_start(
                                out=own_dram[pr][:, q0:q0 + 512],
                                in_=attnT[pr][:, q0:q0 + 512])
                        else:
                            nc.sync.dma_start(
                                out=own3[qb // 2][:, (qb % 2) * 512:
                                                  (qb % 2 + 1) * 512],
                                in_=attnT[pr][:, q0:q0 + 512])
                            if qb % 2 == 1:  # half ready: exchange it now
                                hb = qb // 2
                                nc.gpsimd.collective_compute(
                                    "AllGather", mybir.AluOpType.bypass,
                                    replica_groups=REPLICA_GROUPS,
                                    ins=[own3[hb].opt()],
                                    outs=[all3[hb].opt()])
                                for hf in range(2):
                                    nc.sync.dma_start(
                                        out=atf[:, hf * 4 + pr,
                                                hb * 1024:(hb + 1) * 1024],
                                        in_=all3[hb][hf, :, :])
                    if qstream is not None:  # finish any leftover steps
                        for _ in qstream:
                            pass
                    # pair done: exchange with partner core
                    if pr < NPAIR - 1:
                        nc.gpsimd.collective_compute(
                            "AllGather", mybir.AluOpType.bypass,
                            replica_groups=REPLICA_GROUPS,
                            ins=[own_dram[pr].opt()],
                            outs=[all_dram[pr].opt()])
                        for hf in range(2):
                            nc.sync.dma_start(
                                out=atf[:, hf * 4 + pr, 0:1024],
                                in_=all_dram[pr][hf, :, 0:1024])
                            nc.sync.dma_start(
                                out=atf[:, hf * 4 + pr, 1024:2048],
                                in_=all_dram[pr][hf, :, 1024:2048])

            # ---------------- phase 3: output projection ----------------
            with (
                tc.tile_pool(name="ob", bufs=3) as obp,
                tc.tile_pool(name="fc_ps", bufs=7, space="PSUM") as fcp,
            ):
                # keep the PE's clock warm across the final exchange wait:
                # dependency-free dummy matmuls run while atf is in flight
                warm = fcp.tile([P, 512], F32, tag="warm", bufs=1)
                for _ in range(8):
                    nc.tensor.matmul(warm[:], lhsT=wo_sb[:, 0, 0:P],
                                     rhs=wo_sb[:, 0, :], start=True, stop=True)
                # pair-3 chunks (ic 3, 7) land last; put them at the end of
                # every accumulation chain so earlier chunks' matmuls can run
                # while the final exchange is still in flight
                ic_order = [0, 1, 2, 4, 5, 6, 3, 7]
                for st in range(NT):
                    fc = fcp.tile([P, 512], F32, tag="fc")
                    for i, ic in enumerate(ic_order):
                        nc.tensor.matmul(
                            fc[:],
                            lhsT=atf[:, ic, st * P:(st + 1) * P],
                            rhs=wo_sb[:, ic, :],
                            start=(i == 0), stop=(i == NC - 1))
                    ob = obp.tile([P, 512], F32, tag="ob")
                    if has_bo:
                        nc.vector.tensor_add(out=ob[:], in0=fc[:], in1=bob[:])
                    else:
                        nc.vector.tensor_copy(out=ob[:], in_=fc[:])
                    nc.sync.dma_start(
                        out=out.ap()[st * P:(st + 1) * P, :], in_=ob[:])
            afp.release()

    nc.compile()
    return nc


def _get_nc(flags):
    if flags not in _cache:
        _cache[flags] = _build(*flags)
    return _cache[flags]


def _chunked(xT):
    # [D, cols] -> [128, NC, cols] so each partition's data is contiguous
    cols = xT.shape[1]
    return np.ascontiguousarray(
        xT.reshape(NC, P, cols).transpose(1, 0, 2)).astype(NPBF)


def _prep_inputs(q, k, v, Wq, bq, Wk, bk, Wv, bv, Wo, bo):
    tri = np.triu(np.ones((P, P), np.float32)).astype(NPBF)
    in_maps = []
    for c in range(NCORES):
        b, hf = divmod(c, 2)
        hs = slice(hf * HH, (hf + 1) * HH)
        in_maps.append({
            "qT": _chunked(q[b].T),
            "kT": _chunked(k[b].T),
            "vT": _chunked(v[b].T),
            "wqT": _chunked(Wq[hs].T),
            "wkT": _chunked(Wk[hs].T),
            "wvT": _chunked(Wv[hs].T),
            "woT": _chunked(Wo[hs].T),
            # bq/bk as [128, 4]: partition p, chunk it -> feature it*128+p
            "bq": np.ascontiguousarray(
                np.asarray(bq[hs], np.float32).reshape(4, P).T),
            "bk": np.ascontiguousarray(
                np.asarray(bk[hs], np.float32).reshape(4, P).T),
            "bv": np.asarray(bv[hs], np.float32).reshape(1, HH),
            "bo": np.asarray(bo[hs], np.float32).reshape(1, HH),
            "tri": tri,
        })
    return in_maps


def kernel(q, k, v, mask, Wq, bq, Wk, bk, Wv, bv, Wo, bo, _trace=False):
    q, k, v = (np.asarray(x, np.float32) for x in (q, k, v))
    mask = np.asarray(mask, np.float32)
    exp_mask = np.triu(np.ones((S, S), np.float32), k=1)[None, None]
    assert mask.shape == (1, 1, S, S) and np.array_equal(mask, exp_mask), \
        "kernel specialized for the causal mask produced by setup_inputs()"

    flags = tuple(bool(np.any(np.asarray(x))) for x in (bq, bk, bv, bo))
    nc = _get_nc(flags)
    in_maps = _prep_inputs(q, k, v, Wq, bq, Wk, bk, Wv, bv, Wo, bo)
    res = run_bass_kernel_spmd(nc, in_maps, list(range(NCORES)),
                               trace=_trace)
    if _trace:
        _cache["last_result"] = res
    full = np.empty((B, S, D), np.float32)
    for c in range(NCORES):
        b, hf = divmod(c, 2)
        full[b, :, hf * HH:(hf + 1) * HH] = res.results[c]["out"]
    return full


# revision 36
# speedup vs baseline: 1.2703x; 1.0393x over previous
"""Multi-head attention (B=4, S=2048, D=1024, H=16, causal) on 8 NeuronCores.

Sharding: data-parallel over batch (4) x tensor-parallel over head halves (2).
Core c handles batch c//2 with heads (c%2)*8 .. (c%2)*8+7 and produces output
columns (c%2)*512 .. +512 after an in-pair AllGather of the attention output.

Device pipeline (all bf16 matmuls, fp32 PSUM accumulation):
  phase 1: K and V projections from host-pre-transposed inputs -> KT [i, s]
           per head-pair, V1 [s, (V|ones)] per head with a 64-wide ones block
           so the PV matmul produces softmax denominators on 64 partitions.
  phase 2: per round r: Q projection chunk r (-> QT[r]) immediately followed
           by attention for head pair r, so ScalarE exp work starts ~70us
           earlier and overlaps the remaining projections.  Per head pair /
           512-query block: logits^T tiles [sk=128, sq<=512] via K=64
           row-packed matmul pairs (2 heads concurrent on the PE), exp on
           ScalarE (scale=1/8 fused), causal handled by restricting computed
           ranges + a triangular-mask multiply on the diagonal 128x128 tile,
           PV matmul with full M=128 (64 value cols + 64 ones cols).
           Softmax division uses vector.reciprocal_approx_fast (~5x faster
           than the iterative DVE reciprocal, ~18 correct bits).
  phase 3: pair-wise AllGather of attnT (issued per-pair, overlapped), then
           the output projection for this core's 512 output columns.

Biases are folded in only when nonzero (they are all zero for this model's
inputs): bq/bk via per-partition activation bias on the projection drains,
bv/bo via partition-broadcast tiles added on the V1/output drains.  No PE
cycles are ever spent on biases.
"""

import numpy as np
import ml_dtypes

import concourse.bass as bass
import concourse.mybir as mybir
import concourse.tile as tile
from concourse import bacc
from concourse.bass_utils import run_bass_kernel_spmd

B, S, D, H = 4, 2048, 1024, 16
HD = D // H  # 64
NCORES = 8
HH = D // 2  # 512 = head-half width (8 heads x 64) = output col split
BF16 = mybir.dt.bfloat16
F32 = mybir.dt.float32
NPBF = ml_dtypes.bfloat16

P = 128          # partitions
NB = S // 512    # 4 query/seq blocks of 512
NT = S // P      # 16 seq tiles of 128
NC = D // P      # 8 contraction chunks of 128
NPAIR = 4        # head pairs per core

REPLICA_GROUPS = [[0, 1], [2, 3], [4, 5], [6, 7]]

_cache = {}


def _build(has_bq, has_bk, has_bv, has_bo):
    nc = bacc.Bacc("TRN2", target_bir_lowering=False, debug=False,
                   num_devices=NCORES)

    # ---- dram I/O ----
    qT = nc.dram_tensor("qT", [P, NC, S], BF16, kind="ExternalInput")
    kT = nc.dram_tensor("kT", [P, NC, S], BF16, kind="ExternalInput")
    vT = nc.dram_tensor("vT", [P, NC, S], BF16, kind="ExternalInput")
    wqT = nc.dram_tensor("wqT", [P, NC, HH], BF16, kind="ExternalInput")
    wkT = nc.dram_tensor("wkT", [P, NC, HH], BF16, kind="ExternalInput")
    wvT = nc.dram_tensor("wvT", [P, NC, HH], BF16, kind="ExternalInput")
    woT = nc.dram_tensor("woT", [P, NC, HH], BF16, kind="ExternalInput")
    bqv = nc.dram_tensor("bq", [P, 4], F32, kind="ExternalInput")
    bkv = nc.dram_tensor("bk", [P, 4], F32, kind="ExternalInput")
    bvv = nc.dram_tensor("bv", [1, HH], F32, kind="ExternalInput")
    bov = nc.dram_tensor("bo", [1, HH], F32, kind="ExternalInput")
    trid = nc.dram_tensor("tri", [P, P], BF16, kind="ExternalInput")
    out = nc.dram_tensor("out", [S, HH], F32, kind="ExternalOutput")

    ID = mybir.ActivationFunctionType.Identity

    with tile.TileContext(nc) as tc:
        with (
            tc.tile_pool(name="persist", bufs=1) as pp,
            tc.tile_pool(name="dram", bufs=1, space="DRAM") as dp,
        ):
            # persistent sbuf tensors
            wq_sb = pp.tile([P, NC, HH], BF16, tag="wq")
            wk_sb = pp.tile([P, NC, HH], BF16, tag="wk")
            wv_sb = pp.tile([P, NC, HH], BF16, tag="wv")
            wo_sb = pp.tile([P, NC, HH], BF16, tag="wo")
            bq_sb = pp.tile([P, 4], F32, tag="bq")
            bk_sb = pp.tile([P, 4], F32, tag="bk")
            bv_sb = pp.tile([1, HH], F32, tag="bv") if has_bv else None
            bo_sb = pp.tile([1, HH], F32, tag="bo") if has_bo else None
            # q input chunks live in the persistent pool so their DMAs can be
            # issued right behind kT's (they'd otherwise queue after ~12MB of
            # phase-1 traffic and stall the first Q projection)
            use_xv = not (has_bv or has_bo)  # SBUF headroom needs this off
            if use_xv:
                xq = [pp.tile([P, S], BF16, tag=f"xq{c}", name=f"xq{c}")
                      for c in range(NC)]
            QT = [pp.tile([P, S], BF16, tag=f"qt{p}", name=f"qt{p}")
                  for p in range(NPAIR)]
            KT = [pp.tile([P, S], BF16, tag=f"kt{p}", name=f"kt{p}")
                  for p in range(NPAIR)]
            # V1[s-part, s-tile, head, 128]: every head -> ones in cols 0:64
            # (so PV denominators land on partitions 0:64, where the custom
            # reciprocal op works), V in cols 64:128.
            V1 = pp.tile([P, NT, 8, P], BF16, tag="v1")
            bvb = pp.tile([P, HH], F32, tag="bvb") if has_bv else None
            bob = pp.tile([P, HH], F32, tag="bob") if has_bo else None

            own_dram = [dp.tile([P, S], BF16, tag=f"own{p}", name=f"own{p}")
                        for p in range(NPAIR - 1)]
            all_dram = [dp.tile([2, P, S], BF16, tag=f"all{p}", name=f"all{p}")
                        for p in range(NPAIR - 1)]
            # pair 3's exchange is split in column halves so it overlaps the
            # tail of its own attention instead of serializing before phase 3
            own3 = [dp.tile([P, S // 2], BF16, tag=f"own3{h}", name=f"own3{h}")
                    for h in range(2)]
            all3 = [dp.tile([2, P, S // 2], BF16, tag=f"all3{h}",
                            name=f"all3{h}") for h in range(2)]

            # ------------- phase 1: k/v projections -------------
            with (
                tc.tile_pool(name="xt", bufs=1) as xtp,
                tc.tile_pool(name="proj_ps", bufs=8, space="PSUM") as pps,
            ):
                x_sb = [xtp.tile([P, S], BF16, tag=f"xt{c}", name=f"xt{c}")
                        for c in range(NC)]
                if use_xv:
                    xv_sb = [xtp.tile([P, S], BF16, tag=f"xv{c}",
                                      name=f"xv{c}") for c in range(NC)]
                else:
                    xv_sb = x_sb
                # critical path first: wk + kT chunks feed the first matmuls
                # (kT split in column halves for DMA-ring parallelism)
                for c in range(NC):
                    nc.sync.dma_start(out=wk_sb[:, c, :], in_=wkT.ap()[:, c, :])
                    nc.sync.dma_start(out=x_sb[c][:, 0:1024],
                                      in_=kT.ap()[:, c, 0:1024])
                    nc.sync.dma_start(out=x_sb[c][:, 1024:2048],
                                      in_=kT.ap()[:, c, 1024:2048])
                if use_xv:
                    for c in range(NC):
                        nc.sync.dma_start(out=xq[c][:], in_=qT.ap()[:, c, :])
                biases = [(bq_sb, bqv), (bk_sb, bkv)]
                if has_bv:
                    biases.append((bv_sb, bvv))
                if has_bo:
                    biases.append((bo_sb, bov))
                for b_sb, b_d in biases:
                    nc.sync.dma_start(out=b_sb[:], in_=b_d.ap())
                for c in range(NC):
                    nc.sync.dma_start(out=wv_sb[:, c, :], in_=wvT.ap()[:, c, :])
                    if use_xv:
                        nc.sync.dma_start(out=xv_sb[c][:],
                                          in_=vT.ap()[:, c, :])
                # ones blocks of V1 (written once; V drains only touch V cols)
                nc.vector.memset(V1[:, :, :, 0:64], 1.0)
                if has_bv:
                    nc.gpsimd.partition_broadcast(out=bvb[:], in_=bv_sb[:])
                if has_bo:
                    nc.gpsimd.partition_broadcast(out=bob[:], in_=bo_sb[:])

                # K projection: KT[it] = (Wk x^T)[it*128:(it+1)*128, :]
                for it in range(4):
                    ps = [pps.tile([P, 512], F32, tag="proj", name=f"proj{sb}")
                          for sb in range(4)]
                    for c in range(NC):
                        for sb in range(4):
                            nc.tensor.matmul(
                                ps[sb][:],
                                lhsT=wk_sb[:, c, it * P:(it + 1) * P],
                                rhs=x_sb[c][:, sb * 512:(sb + 1) * 512],
                                start=(c == 0), stop=(c == NC - 1))
                    for sb in range(4):
                        dst = KT[it][:, sb * 512:(sb + 1) * 512]
                        if has_bk:
                            nc.scalar.activation(
                                out=dst, in_=ps[sb][:], func=ID,
                                bias=bk_sb[:, it:it + 1])
                        else:
                            nc.scalar.copy(out=dst, in_=ps[sb][:])

                # V projection: V1[:, st, h, vcols] = (x^T)^T Wv per seq tile
                if not use_xv:
                    for c in range(NC):
                        nc.sync.dma_start(out=xv_sb[c][:],
                                          in_=vT.ap()[:, c, :])
                for c in range(NC):
                    nc.sync.dma_start(out=wq_sb[:, c, :], in_=wqT.ap()[:, c, :])
                    nc.sync.dma_start(out=wo_sb[:, c, :], in_=woT.ap()[:, c, :])
                for it in range(4):
                    ps = [pps.tile([P, 512], F32, tag="proj", name=f"proj{sb}")
                          for sb in range(4)]
                    for c in range(NC):
                        for sb in range(4):
                            st = it * 4 + sb
                            nc.tensor.matmul(
                                ps[sb][:],
                                lhsT=xv_sb[c][:, st * P:(st + 1) * P],
                                rhs=wv_sb[:, c, :],
                                start=(c == 0), stop=(c == NC - 1))
                    for sb in range(4):
                        st = it * 4 + sb
                        pv3 = ps[sb][:].rearrange("p (h d) -> p h d", h=8)
                        if has_bv:
                            bv3 = bvb[:].rearrange("p (h d) -> p h d", h=8)
                            nc.vector.tensor_add(
                                out=V1[:, st, :, 64:128],
                                in0=pv3[:, :, :], in1=bv3[:, :, :])
                        else:
                            nc.scalar.copy(
                                out=V1[:, st, :, 64:128],
                                in_=pv3[:, :, :])

            # atf lives only in phases 2+3; freeing its 32KB/partition during
            # phase 1 makes room for the separate K/V input tile sets
            afp = tc.alloc_tile_pool(name="atfp", bufs=1)
            atf = afp.tile([P, NC, S], BF16, tag="atf")

            # ------ phase 2: q projections interleaved with attention ------
            # qproj(pr+1) matmuls are dripped into attention(pr)'s chunk
            # stream (one step per chunk) so ScalarE's exp stream never
            # starves while the PE does projection work.
            with (
                tc.tile_pool(name="xq", bufs=1) as xqp,
                tc.tile_pool(name="pt", bufs=6) as ptp,
                tc.tile_pool(name="rec", bufs=2) as rcp,
                tc.tile_pool(name="lg_ps", bufs=2, space="PSUM") as lgp,
                tc.tile_pool(name="pv_ps", bufs=2, space="PSUM") as pvp,
                tc.tile_pool(name="qp_ps", bufs=2, space="PSUM") as qps,
            ):
                if not use_xv:
                    xq = [xqp.tile([P, S], BF16, tag=f"xq{c}", name=f"xq{c}")
                          for c in range(NC)]
                    for c in range(NC):
                        nc.sync.dma_start(out=xq[c][:], in_=qT.ap()[:, c, :])
                tri_sb = rcp.tile([P, P], BF16, tag="tri", bufs=1)
                nc.sync.dma_start(out=tri_sb[:], in_=trid.ap())

                def qproj_steps(pr):
                    # Q projection chunk pr -> QT[pr], one emitted op/step
                    for sb in range(4):
                        qp = qps.tile([P, 512], F32, tag="qp")
                        for c in range(NC):
                            nc.tensor.matmul(
                                qp[:],
                                lhsT=wq_sb[:, c, pr * P:(pr + 1) * P],
                                rhs=xq[c][:, sb * 512:(sb + 1) * 512],
                                start=(c == 0), stop=(c == NC - 1))
                            yield
                        dst = QT[pr][:, sb * 512:(sb + 1) * 512]
                        if has_bq:
                            nc.scalar.activation(
                                out=dst, in_=qp[:], func=ID,
                                bias=bq_sb[:, pr:pr + 1])
                        else:
                            nc.scalar.copy(out=dst, in_=qp[:])
                        yield

                for _ in qproj_steps(0):  # pair 0's projection up front
                    pass

                for pr in range(NPAIR):
                    qstream = qproj_steps(pr + 1) if pr + 1 < NPAIR else None
                    for qb in range(NB):
                        pvA = pvp.tile([P, 512], F32, tag="pv")
                        pvB = pvp.tile([P, 512], F32, tag="pv")
                        nch = qb * 4 + 4
                        q0 = qb * 512

                        def emit_pv(c, pt, off, pvA=pvA, pvB=pvB, pr=pr,
                                    nch=nch):
                            nc.tensor.matmul(
                                pvA[:, off:512],
                                lhsT=V1[:, c, 2 * pr, :],
                                rhs=pt[:, off:512],
                                start=(c == 0), stop=(c == nch - 1))
                            nc.tensor.matmul(
                                pvB[:, off:512],
                                lhsT=V1[:, c, 2 * pr + 1, :],
                                rhs=pt[:, 512 + off:1024],
                                start=(c == 0), stop=(c == nch - 1))

                        pending = []  # (c, pt, off) awaiting PV emission
                        for c in range(nch):
                            jj = c - qb * 4
                            off = 128 * jj if jj > 0 else 0
                            lg = lgp.tile([P, 1024], F32, tag="lg")
                            nc.tensor.matmul(
                                lg[:, off:512],
                                lhsT=KT[pr][0:64, c * P:(c + 1) * P],
                                rhs=QT[pr][0:64, q0 + off:q0 + 512],
                                start=True, stop=True, tile_position=(0, 0))
                            nc.tensor.matmul(
                                lg[:, 512 + off:1024],
                                lhsT=KT[pr][64:128, c * P:(c + 1) * P],
                                rhs=QT[pr][64:128, q0 + off:q0 + 512],
                                start=True, stop=True, tile_position=(64, 0))
                            pt = ptp.tile([P, 1024], BF16, tag="pt")
                            lg3 = lg[:].rearrange("p (h n) -> p h n", h=2)
                            pt3 = pt[:].rearrange("p (h n) -> p h n", h=2)
                            nc.scalar.activation(
                                out=pt3[:, :, off:512],
                                in_=lg3[:, :, off:512],
                                func=mybir.ActivationFunctionType.Exp,
                                scale=0.125)
                            if jj >= 0:  # diagonal 128x128: causal mask
                                nc.vector.tensor_mul(
                                    out=pt[:, off:off + P],
                                    in0=pt[:, off:off + P], in1=tri_sb[:])
                                nc.vector.tensor_mul(
                                    out=pt[:, 512 + off:512 + off + P],
                                    in0=pt[:, 512 + off:512 + off + P],
                                    in1=tri_sb[:])
                            if qstream is not None:
                                next(qstream, None)
                            pending.append((c, pt, off))
                            if len(pending) > 3:
                                emit_pv(*pending.pop(0))
                        for args in pending:
                            emit_pv(*args)
                        # drain: denominators on partitions 0:64 (ones cols),
                        # values on 64:128 for both heads.
                        rec = rcp.tile([P, 1024], F32, tag="rec")
                        stage = rcp.tile([P, 512], BF16, tag="ats", bufs=3)
                        nc.vector.reciprocal_approx_fast(
                            rec[0:64, 0:512], pvA[0:64, :])
                        nc.vector.reciprocal_approx_fast(
                            rec[0:64, 512:1024], pvB[0:64, :])
                        nc.vector.tensor_mul(
                            out=stage[0:64, :],
                            in0=pvA[64:128, :], in1=rec[0:64, 0:512])
                        nc.vector.tensor_mul(
                            out=stage[64:128, :],
                            in0=pvB[64:128, :], in1=rec[0:64, 512:1024])
                        # stream this query block out to DRAM immediately so
                        # the pair's exchange isn't gated on one big DMA
                        if pr < NPAIR - 1:
                            nc.sync.dma_start(
                                out=own_dram[pr][:, q0:q0 + 512],
                                in_=stage[:])
                        else:
                            nc.sync.dma_start(
                                out=own3[qb // 2][:, (qb % 2) * 512:
                                                  (qb % 2 + 1) * 512],
                                in_=stage[:])
                            if qb % 2 == 1:  # half ready: exchange it now
                                hb = qb // 2
                                nc.gpsimd.collective_compute(
                                    "AllGather", mybir.AluOpType.bypass,
                                    replica_groups=REPLICA_GROUPS,
                                    ins=[own3[hb].opt()],
                                    outs=[all3[hb].opt()])
                                for hf in range(2):
                                    nc.sync.dma_start(
                                        out=atf[:, hf * 4 + pr,
                                                hb * 1024:(hb + 1) * 1024],
                                        in_=all3[hb][hf, :, :])
                    if qstream is not None:  # finish any leftover steps
                        for _ in qstream:
                            pass
                    # pair done: exchange with partner core
                    if pr < NPAIR - 1:
                        nc.gpsimd.collective_compute(
                            "AllGather", mybir.AluOpType.bypass,
                            replica_groups=REPLICA_GROUPS,
                            ins=[own_dram[pr].opt()],
                            outs=[all_dram[pr].opt()])
                        for hf in range(2):
                            nc.sync.dma_start(
                                out=atf[:, hf * 4 + pr, 0:1024],
                                in_=all_dram[pr][hf, :, 0:1024])
                            nc.sync.dma_start(
                                out=atf[:, hf * 4 + pr, 1024:2048],
                                in_=all_dram[pr][hf, :, 1024:2048])

            # ---------------- phase 3: output projection ----------------
            with (
                tc.tile_pool(name="ob", bufs=3) as obp,
                tc.tile_pool(name="fc_ps", bufs=7, space="PSUM") as fcp,
            ):
                # keep the PE's clock warm across the final exchange wait:
                # dependency-free dummy matmuls run while atf is in flight
                warm = fcp.tile([P, 512], F32, tag="warm", bufs=1)
                for _ in range(8):
                    nc.tensor.matmul(warm[:], lhsT=wo_sb[:, 0, 0:P],
                                     rhs=wo_sb[:, 0, :], start=True, stop=True)
                # pair-3 chunks (ic 3, 7) land last; put them at the end of
                # every accumulation chain so earlier chunks' matmuls can run
                # while the final exchange is still in flight
                ic_order = [0, 1, 2, 4, 5, 6, 3, 7]
                for st in range(NT):
                    fc = fcp.tile([P, 512], F32, tag="fc")
                    for i, ic in enumerate(ic_order):
                        nc.tensor.matmul(
                            fc[:],
                            lhsT=atf[:, ic, st * P:(st + 1) * P],
                            rhs=wo_sb[:, ic, :],
                            start=(i == 0), stop=(i == NC - 1))
                    ob = obp.tile([P, 512], F32, tag="ob")
                    if has_bo:
                        nc.vector.tensor_add(out=ob[:], in0=fc[:], in1=bob[:])
                    else:
                        nc.vector.tensor_copy(out=ob[:], in_=fc[:])
                    nc.sync.dma_start(
                        out=out.ap()[st * P:(st + 1) * P, :], in_=ob[:])
            afp.release()

    nc.compile()
    return nc


def _get_nc(flags):
    if flags not in _cache:
        _cache[flags] = _build(*flags)
    return _cache[flags]


def _chunked(xT):
    # [D, cols] -> [128, NC, cols] so each partition's data is contiguous
    cols = xT.shape[1]
    return np.ascontiguousarray(
        xT.reshape(NC, P, cols).transpose(1, 0, 2)).astype(NPBF)


def _prep_inputs(q, k, v, Wq, bq, Wk, bk, Wv, bv, Wo, bo):
    tri = np.triu(np.ones((P, P), np.float32)).astype(NPBF)
    in_maps = []
    for c in range(NCORES):
        b, hf = divmod(c, 2)
        hs = slice(hf * HH, (hf + 1) * HH)
        in_maps.append({
            "qT": _chunked(q[b].T),
            "kT": _chunked(k[b].T),
            "vT": _chunked(v[b].T),
            "wqT": _chunked(Wq[hs].T),
            "wkT": _chunked(Wk[hs].T),
            "wvT": _chunked(Wv[hs].T),
            "woT": _chunked(Wo[hs].T),
            # bq/bk as [128, 4]: partition p, chunk it -> feature it*128+p
            "bq": np.ascontiguousarray(
                np.asarray(bq[hs], np.float32).reshape(4, P).T),
            "bk": np.ascontiguousarray(
                np.asarray(bk[hs], np.float32).reshape(4, P).T),
            "bv": np.asarray(bv[hs], np.float32).reshape(1, HH),
            "bo": np.asarray(bo[hs], np.float32).reshape(1, HH),
            "tri": tri,
        })
    return in_maps


def kernel(q, k, v, mask, Wq, bq, Wk, bk, Wv, bv, Wo, bo, _trace=False):
    q, k, v = (np.asarray(x, np.float32) for x in (q, k, v))
    mask = np.asarray(mask, np.float32)
    exp_mask = np.triu(np.ones((S, S), np.float32), k=1)[None, None]
    assert mask.shape == (1, 1, S, S) and np.array_equal(mask, exp_mask), \
        "kernel specialized for the causal mask produced by setup_inputs()"

    flags = tuple(bool(np.any(np.asarray(x))) for x in (bq, bk, bv, bo))
    nc = _get_nc(flags)
    in_maps = _prep_inputs(q, k, v, Wq, bq, Wk, bk, Wv, bv, Wo, bo)
    res = run_bass_kernel_spmd(nc, in_maps, list(range(NCORES)),
                               trace=_trace)
    if _trace:
        _cache["last_result"] = res
    full = np.empty((B, S, D), np.float32)
    for c in range(NCORES):
        b, hf = divmod(c, 2)
        full[b, :, hf * HH:(hf + 1) * HH] = res.results[c]["out"]
    return full


# revision 38
# speedup vs baseline: 1.2863x; 1.0126x over previous
"""Multi-head attention (B=4, S=2048, D=1024, H=16, causal) on 8 NeuronCores.

Sharding: data-parallel over batch (4) x tensor-parallel over head halves (2).
Core c handles batch c//2 with heads (c%2)*8 .. (c%2)*8+7 and produces output
columns (c%2)*512 .. +512 after an in-pair AllGather of the attention output.

Device pipeline (all bf16 matmuls, fp32 PSUM accumulation):
  phase 1: K and V projections from host-pre-transposed inputs -> KT [i, s]
           per head-pair, V1 [s, (V|ones)] per head with a 64-wide ones block
           so the PV matmul produces softmax denominators on 64 partitions.
  phase 2: per round r: Q projection chunk r (-> QT[r]) immediately followed
           by attention for head pair r, so ScalarE exp work starts ~70us
           earlier and overlaps the remaining projections.  Per head pair /
           512-query block: logits^T tiles [sk=128, sq<=512] via K=64
           row-packed matmul pairs (2 heads concurrent on the PE), exp on
           ScalarE (scale=1/8 fused), causal handled by restricting computed
           ranges + a triangular-mask multiply on the diagonal 128x128 tile,
           PV matmul with full M=128 (64 value cols + 64 ones cols).
           Softmax division uses vector.reciprocal_approx_fast (~5x faster
           than the iterative DVE reciprocal, ~18 correct bits).
  phase 3: pair-wise AllGather of attnT (issued per-pair, overlapped), then
           the output projection for this core's 512 output columns.

Biases are folded in only when nonzero (they are all zero for this model's
inputs): bq/bk via per-partition activation bias on the projection drains,
bv/bo via partition-broadcast tiles added on the V1/output drains.  No PE
cycles are ever spent on biases.
"""

import numpy as np
import ml_dtypes

import concourse.bass as bass
import concourse.mybir as mybir
import concourse.tile as tile
from concourse import bacc
from concourse.bass_utils import run_bass_kernel_spmd

B, S, D, H = 4, 2048, 1024, 16
HD = D // H  # 64
NCORES = 8
HH = D // 2  # 512 = head-half width (8 heads x 64) = output col split
BF16 = mybir.dt.bfloat16
F32 = mybir.dt.float32
NPBF = ml_dtypes.bfloat16

P = 128          # partitions
NB = S // 512    # 4 query/seq blocks of 512
NT = S // P      # 16 seq tiles of 128
NC = D // P      # 8 contraction chunks of 128
NPAIR = 4        # head pairs per core

REPLICA_GROUPS = [[0, 1], [2, 3], [4, 5], [6, 7]]

_cache = {}


def _build(has_bq, has_bk, has_bv, has_bo):
    nc = bacc.Bacc("TRN2", target_bir_lowering=False, debug=False,
                   num_devices=NCORES)

    # ---- dram I/O ----
    qT = nc.dram_tensor("qT", [P, NC, S], BF16, kind="ExternalInput")
    kT = nc.dram_tensor("kT", [P, NC, S], BF16, kind="ExternalInput")
    vT = nc.dram_tensor("vT", [P, NC, S], BF16, kind="ExternalInput")
    wqT = nc.dram_tensor("wqT", [P, NC, HH], BF16, kind="ExternalInput")
    wkT = nc.dram_tensor("wkT", [P, NC, HH], BF16, kind="ExternalInput")
    wvT = nc.dram_tensor("wvT", [P, NC, HH], BF16, kind="ExternalInput")
    woT = nc.dram_tensor("woT", [P, NC, HH], BF16, kind="ExternalInput")
    bqv = nc.dram_tensor("bq", [P, 4], F32, kind="ExternalInput")
    bkv = nc.dram_tensor("bk", [P, 4], F32, kind="ExternalInput")
    bvv = nc.dram_tensor("bv", [1, HH], F32, kind="ExternalInput")
    bov = nc.dram_tensor("bo", [1, HH], F32, kind="ExternalInput")
    trid = nc.dram_tensor("tri", [P, P], BF16, kind="ExternalInput")
    out = nc.dram_tensor("out", [S, HH], F32, kind="ExternalOutput")

    ID = mybir.ActivationFunctionType.Identity

    with tile.TileContext(nc) as tc:
        with (
            tc.tile_pool(name="persist", bufs=1) as pp,
            tc.tile_pool(name="dram", bufs=1, space="DRAM") as dp,
        ):
            # persistent sbuf tensors
            wq_sb = pp.tile([P, NC, HH], BF16, tag="wq")
            wk_sb = pp.tile([P, NC, HH], BF16, tag="wk")
            wv_sb = pp.tile([P, NC, HH], BF16, tag="wv")
            wo_sb = pp.tile([P, NC, HH], BF16, tag="wo")
            bq_sb = pp.tile([P, 4], F32, tag="bq")
            bk_sb = pp.tile([P, 4], F32, tag="bk")
            bv_sb = pp.tile([1, HH], F32, tag="bv") if has_bv else None
            bo_sb = pp.tile([1, HH], F32, tag="bo") if has_bo else None
            # q input chunks live in the persistent pool so their DMAs can be
            # issued right behind kT's (they'd otherwise queue after ~12MB of
            # phase-1 traffic and stall the first Q projection)
            use_xv = not (has_bv or has_bo)  # SBUF headroom needs this off
            if use_xv:
                xq = [pp.tile([P, S], BF16, tag=f"xq{c}", name=f"xq{c}")
                      for c in range(NC)]
            QT = [pp.tile([P, S], BF16, tag=f"qt{p}", name=f"qt{p}")
                  for p in range(NPAIR)]
            KT = [pp.tile([P, S], BF16, tag=f"kt{p}", name=f"kt{p}")
                  for p in range(NPAIR)]
            # V1[s-part, s-tile, head, 128]: every head -> ones in cols 0:64
            # (so PV denominators land on partitions 0:64, where the custom
            # reciprocal op works), V in cols 64:128.
            V1 = pp.tile([P, NT, 8, P], BF16, tag="v1")
            bvb = pp.tile([P, HH], F32, tag="bvb") if has_bv else None
            bob = pp.tile([P, HH], F32, tag="bob") if has_bo else None

            own_dram = [dp.tile([P, S], BF16, tag=f"own{p}", name=f"own{p}")
                        for p in range(NPAIR - 1)]
            all_dram = [dp.tile([2, P, S], BF16, tag=f"all{p}", name=f"all{p}")
                        for p in range(NPAIR - 1)]
            # pair 3's exchange is split in column halves so it overlaps the
            # tail of its own attention instead of serializing before phase 3
            own3 = [dp.tile([P, S // 2], BF16, tag=f"own3{h}", name=f"own3{h}")
                    for h in range(2)]
            all3 = [dp.tile([2, P, S // 2], BF16, tag=f"all3{h}",
                            name=f"all3{h}") for h in range(2)]

            # ------------- phase 1: k/v projections -------------
            with (
                tc.tile_pool(name="xt", bufs=1) as xtp,
                tc.tile_pool(name="proj_ps", bufs=8, space="PSUM") as pps,
            ):
                x_sb = [xtp.tile([P, S], BF16, tag=f"xt{c}", name=f"xt{c}")
                        for c in range(NC)]
                if use_xv:
                    xv_sb = [xtp.tile([P, S], BF16, tag=f"xv{c}",
                                      name=f"xv{c}") for c in range(NC)]
                else:
                    xv_sb = x_sb
                # critical path first: wk + kT chunks feed the first matmuls
                # (kT split in column halves for DMA-ring parallelism)
                for c in range(NC):
                    nc.sync.dma_start(out=wk_sb[:, c, :], in_=wkT.ap()[:, c, :])
                    nc.sync.dma_start(out=x_sb[c][:, 0:1024],
                                      in_=kT.ap()[:, c, 0:1024])
                    nc.sync.dma_start(out=x_sb[c][:, 1024:2048],
                                      in_=kT.ap()[:, c, 1024:2048])
                if use_xv:
                    for c in range(NC):
                        nc.sync.dma_start(out=xq[c][:], in_=qT.ap()[:, c, :])
                biases = [(bq_sb, bqv), (bk_sb, bkv)]
                if has_bv:
                    biases.append((bv_sb, bvv))
                if has_bo:
                    biases.append((bo_sb, bov))
                for b_sb, b_d in biases:
                    nc.sync.dma_start(out=b_sb[:], in_=b_d.ap())
                for c in range(NC):
                    nc.sync.dma_start(out=wv_sb[:, c, :], in_=wvT.ap()[:, c, :])
                    if use_xv:
                        nc.sync.dma_start(out=xv_sb[c][:],
                                          in_=vT.ap()[:, c, :])
                # ones blocks of V1 (written once; V drains only touch V cols)
                nc.vector.memset(V1[:, :, :, 0:64], 1.0)
                if has_bv:
                    nc.gpsimd.partition_broadcast(out=bvb[:], in_=bv_sb[:])
                if has_bo:
                    nc.gpsimd.partition_broadcast(out=bob[:], in_=bo_sb[:])

                # K projection: KT[it] = (Wk x^T)[it*128:(it+1)*128, :]
                for it in range(4):
                    ps = [pps.tile([P, 512], F32, tag="proj", name=f"proj{sb}")
                          for sb in range(4)]
                    for c in range(NC):
                        for sb in range(4):
                            nc.tensor.matmul(
                                ps[sb][:],
                                lhsT=wk_sb[:, c, it * P:(it + 1) * P],
                                rhs=x_sb[c][:, sb * 512:(sb + 1) * 512],
                                start=(c == 0), stop=(c == NC - 1))
                    for sb in range(4):
                        dst = KT[it][:, sb * 512:(sb + 1) * 512]
                        if has_bk:
                            nc.scalar.activation(
                                out=dst, in_=ps[sb][:], func=ID,
                                bias=bk_sb[:, it:it + 1])
                        else:
                            nc.scalar.copy(out=dst, in_=ps[sb][:])

                # V projection: V1[:, st, h, vcols] = (x^T)^T Wv per seq tile
                if not use_xv:
                    for c in range(NC):
                        nc.sync.dma_start(out=xv_sb[c][:],
                                          in_=vT.ap()[:, c, :])
                for c in range(NC):
                    nc.sync.dma_start(out=wq_sb[:, c, :], in_=wqT.ap()[:, c, :])
                    nc.sync.dma_start(out=wo_sb[:, c, :], in_=woT.ap()[:, c, :])
                for it in range(4):
                    ps = [pps.tile([P, 512], F32, tag="proj", name=f"proj{sb}")
                          for sb in range(4)]
                    for c in range(NC):
                        for sb in range(4):
                            st = it * 4 + sb
                            nc.tensor.matmul(
                                ps[sb][:],
                                lhsT=xv_sb[c][:, st * P:(st + 1) * P],
                                rhs=wv_sb[:, c, :],
                                start=(c == 0), stop=(c == NC - 1))
                    for sb in range(4):
                        st = it * 4 + sb
                        pv3 = ps[sb][:].rearrange("p (h d) -> p h d", h=8)
                        if has_bv:
                            bv3 = bvb[:].rearrange("p (h d) -> p h d", h=8)
                            nc.vector.tensor_add(
                                out=V1[:, st, :, 64:128],
                                in0=pv3[:, :, :], in1=bv3[:, :, :])
                        else:
                            nc.scalar.copy(
                                out=V1[:, st, :, 64:128],
                                in_=pv3[:, :, :])

            # atf lives only in phases 2+3; freeing its 32KB/partition during
            # phase 1 makes room for the separate K/V input tile sets
            afp = tc.alloc_tile_pool(name="atfp", bufs=1)
            atf = afp.tile([P, NC, S], BF16, tag="atf")

            # ------ phase 2: q projections interleaved with attention ------
            # qproj(pr+1) matmuls are dripped into attention(pr)'s chunk
            # stream (one step per chunk) so ScalarE's exp stream never
            # starves while the PE does projection work.
            with (
                tc.tile_pool(name="xq", bufs=1) as xqp,
                tc.tile_pool(name="pt", bufs=6) as ptp,
                tc.tile_pool(name="rec", bufs=2) as rcp,
                tc.tile_pool(name="lg_ps", bufs=2, space="PSUM") as lgp,
                tc.tile_pool(name="pv_ps", bufs=2, space="PSUM") as pvp,
                tc.tile_pool(name="qp_ps", bufs=2, space="PSUM") as qps,
            ):
                if not use_xv:
                    xq = [xqp.tile([P, S], BF16, tag=f"xq{c}", name=f"xq{c}")
                          for c in range(NC)]
                    for c in range(NC):
                        nc.sync.dma_start(out=xq[c][:], in_=qT.ap()[:, c, :])
                tri_sb = rcp.tile([P, P], BF16, tag="tri", bufs=1)
                nc.sync.dma_start(out=tri_sb[:], in_=trid.ap())

                def qproj_steps(pr):
                    # Q projection chunk pr -> QT[pr], one emitted op/step
                    for sb in range(4):
                        qp = qps.tile([P, 512], F32, tag="qp")
                        for c in range(NC):
                            nc.tensor.matmul(
                                qp[:],
                                lhsT=wq_sb[:, c, pr * P:(pr + 1) * P],
                                rhs=xq[c][:, sb * 512:(sb + 1) * 512],
                                start=(c == 0), stop=(c == NC - 1))
                            yield
                        dst = QT[pr][:, sb * 512:(sb + 1) * 512]
                        if has_bq:
                            nc.scalar.activation(
                                out=dst, in_=qp[:], func=ID,
                                bias=bq_sb[:, pr:pr + 1])
                        else:
                            nc.scalar.copy(out=dst, in_=qp[:])
                        yield

                for _ in qproj_steps(0):  # pair 0's projection up front
                    pass

                # PV matmuls and softmax drains trail the logits/exp stream
                # through a FIFO that persists across query-block and pair
                # boundaries, so the PE never serializes a drain in front of
                # the next block's logits (which would starve ScalarE).
                tail_ops = []  # (is_pv, closure)

                def push(is_pv, fn):
                    tail_ops.append((is_pv, fn))
                    while sum(1 for p, _ in tail_ops if p) > 3:
                        tail_ops.pop(0)[1]()

                for pr in range(NPAIR):
                    qstream = qproj_steps(pr + 1) if pr + 1 < NPAIR else None
                    for qb in range(NB):
                        nch = qb * 4 + 4
                        q0 = qb * 512
                        pv = {}  # allocated lazily at first PV emission

                        def emit_pv(c, pt, off, pv=pv, pr=pr, nch=nch):
                            if c == 0:
                                pv["A"] = pvp.tile([P, 512], F32, tag="pv",
                                                   name="pvA")
                                pv["B"] = pvp.tile([P, 512], F32, tag="pv",
                                                   name="pvB")
                            nc.tensor.matmul(
                                pv["A"][:, off:512],
                                lhsT=V1[:, c, 2 * pr, :],
                                rhs=pt[:, off:512],
                                start=(c == 0), stop=(c == nch - 1))
                            nc.tensor.matmul(
                                pv["B"][:, off:512],
                                lhsT=V1[:, c, 2 * pr + 1, :],
                                rhs=pt[:, 512 + off:1024],
                                start=(c == 0), stop=(c == nch - 1))

                        def drain(pv=pv, q0=q0, qb=qb, pr=pr):
                            pvA, pvB = pv["A"], pv["B"]
                            rec = rcp.tile([P, 1024], F32, tag="rec")
                            stage = rcp.tile([P, 512], BF16, tag="ats",
                                             bufs=3)
                            nc.vector.reciprocal_approx_fast(
                                rec[0:64, 0:512], pvA[0:64, :])
                            nc.vector.reciprocal_approx_fast(
                                rec[0:64, 512:1024], pvB[0:64, :])
                            nc.vector.tensor_mul(
                                out=stage[0:64, :],
                                in0=pvA[64:128, :], in1=rec[0:64, 0:512])
                            nc.vector.tensor_mul(
                                out=stage[64:128, :],
                                in0=pvB[64:128, :], in1=rec[0:64, 512:1024])
                            if pr < NPAIR - 1:
                                nc.sync.dma_start(
                                    out=own_dram[pr][:, q0:q0 + 512],
                                    in_=stage[:])
                                if qb == NB - 1:  # pair complete: exchange
                                    nc.gpsimd.collective_compute(
                                        "AllGather", mybir.AluOpType.bypass,
                                        replica_groups=REPLICA_GROUPS,
                                        ins=[own_dram[pr].opt()],
                                        outs=[all_dram[pr].opt()])
                                    for hf in range(2):
                                        nc.sync.dma_start(
                                            out=atf[:, hf * 4 + pr, 0:1024],
                                            in_=all_dram[pr][hf, :, 0:1024])
                                        nc.sync.dma_start(
                                            out=atf[:, hf * 4 + pr,
                                                    1024:2048],
                                            in_=all_dram[pr][hf, :,
                                                             1024:2048])
                            else:
                                nc.sync.dma_start(
                                    out=own3[qb // 2][:, (qb % 2) * 512:
                                                      (qb % 2 + 1) * 512],
                                    in_=stage[:])
                                if qb % 2 == 1:  # half ready: exchange it
                                    hb = qb // 2
                                    nc.gpsimd.collective_compute(
                                        "AllGather", mybir.AluOpType.bypass,
                                        replica_groups=REPLICA_GROUPS,
                                        ins=[own3[hb].opt()],
                                        outs=[all3[hb].opt()])
                                    for hf in range(2):
                                        nc.sync.dma_start(
                                            out=atf[:, hf * 4 + pr,
                                                    hb * 1024:
                                                    (hb + 1) * 1024],
                                            in_=all3[hb][hf, :, :])

                        for c in range(nch):
                            jj = c - qb * 4
                            off = 128 * jj if jj > 0 else 0
                            lg = lgp.tile([P, 1024], F32, tag="lg")
                            nc.tensor.matmul(
                                lg[:, off:512],
                                lhsT=KT[pr][0:64, c * P:(c + 1) * P],
                                rhs=QT[pr][0:64, q0 + off:q0 + 512],
                                start=True, stop=True, tile_position=(0, 0))
                            nc.tensor.matmul(
                                lg[:, 512 + off:1024],
                                lhsT=KT[pr][64:128, c * P:(c + 1) * P],
                                rhs=QT[pr][64:128, q0 + off:q0 + 512],
                                start=True, stop=True, tile_position=(64, 0))
                            pt = ptp.tile([P, 1024], BF16, tag="pt")
                            lg3 = lg[:].rearrange("p (h n) -> p h n", h=2)
                            pt3 = pt[:].rearrange("p (h n) -> p h n", h=2)
                            nc.scalar.activation(
                                out=pt3[:, :, off:512],
                                in_=lg3[:, :, off:512],
                                func=mybir.ActivationFunctionType.Exp,
                                scale=0.125)
                            if jj >= 0:  # diagonal 128x128: causal mask
                                nc.vector.tensor_mul(
                                    out=pt[:, off:off + P],
                                    in0=pt[:, off:off + P], in1=tri_sb[:])
                                nc.vector.tensor_mul(
                                    out=pt[:, 512 + off:512 + off + P],
                                    in0=pt[:, 512 + off:512 + off + P],
                                    in1=tri_sb[:])
                            if qstream is not None:
                                next(qstream, None)
                            push(True, lambda c=c, pt=pt, off=off,
                                 f=emit_pv: f(c, pt, off))
                        push(False, drain)
                    if qstream is not None:  # finish any leftover steps
                        for _ in qstream:
                            pass
                for _, fn in tail_ops:  # flush the last PVs + drains
                    fn()
                tail_ops.clear()

            # ---------------- phase 3: output projection ----------------
            with (
                tc.tile_pool(name="ob", bufs=3) as obp,
                tc.tile_pool(name="fc_ps", bufs=7, space="PSUM") as fcp,
            ):
                # keep the PE's clock warm across the final exchange wait:
                # dependency-free dummy matmuls run while atf is in flight
                warm = fcp.tile([P, 512], F32, tag="warm", bufs=1)
                for _ in range(16):
                    nc.tensor.matmul(warm[:, 0:P], lhsT=wo_sb[:, 0, 0:P],
                                     rhs=wo_sb[:, 0, 0:P],
                                     start=True, stop=True)
                # pair-3 chunks (ic 3, 7) land last; put them at the end of
                # every accumulation chain so earlier chunks' matmuls can run
                # while the final exchange is still in flight
                ic_order = [0, 1, 2, 4, 5, 6, 3, 7]
                for st in range(NT):
                    fc = fcp.tile([P, 512], F32, tag="fc")
                    for i, ic in enumerate(ic_order):
                        nc.tensor.matmul(
                            fc[:],
                            lhsT=atf[:, ic, st * P:(st + 1) * P],
                            rhs=wo_sb[:, ic, :],
                            start=(i == 0), stop=(i == NC - 1))
                    ob = obp.tile([P, 512], F32, tag="ob")
                    if has_bo:
                        nc.vector.tensor_add(out=ob[:], in0=fc[:], in1=bob[:])
                    else:
                        nc.vector.tensor_copy(out=ob[:], in_=fc[:])
                    nc.sync.dma_start(
                        out=out.ap()[st * P:(st + 1) * P, :], in_=ob[:])
            afp.release()

    nc.compile()
    return nc


def _get_nc(flags):
    if flags not in _cache:
        _cache[flags] = _build(*flags)
    return _cache[flags]


def _chunked(xT):
    # [D, cols] -> [128, NC, cols] so each partition's data is contiguous
    cols = xT.shape[1]
    return np.ascontiguousarray(
        xT.reshape(NC, P, cols).transpose(1, 0, 2)).astype(NPBF)


def _prep_inputs(q, k, v, Wq, bq, Wk, bk, Wv, bv, Wo, bo):
    tri = np.triu(np.ones((P, P), np.float32)).astype(NPBF)
    in_maps = []
    for c in range(NCORES):
        b, hf = divmod(c, 2)
        hs = slice(hf * HH, (hf + 1) * HH)
        in_maps.append({
            "qT": _chunked(q[b].T),
            "kT": _chunked(k[b].T),
            "vT": _chunked(v[b].T),
            "wqT": _chunked(Wq[hs].T),
            "wkT": _chunked(Wk[hs].T),
            "wvT": _chunked(Wv[hs].T),
            "woT": _chunked(Wo[hs].T),
            # bq/bk as [128, 4]: partition p, chunk it -> feature it*128+p
            "bq": np.ascontiguousarray(
                np.asarray(bq[hs], np.float32).reshape(4, P).T),
            "bk": np.ascontiguousarray(
                np.asarray(bk[hs], np.float32).reshape(4, P).T),
            "bv": np.asarray(bv[hs], np.float32).reshape(1, HH),
            "bo": np.asarray(bo[hs], np.float32).reshape(1, HH),
            "tri": tri,
        })
    return in_maps


def kernel(q, k, v, mask, Wq, bq, Wk, bk, Wv, bv, Wo, bo, _trace=False):
    q, k, v = (np.asarray(x, np.float32) for x in (q, k, v))
    mask = np.asarray(mask, np.float32)
    exp_mask = np.triu(np.ones((S, S), np.float32), k=1)[None, None]
    assert mask.shape == (1, 1, S, S) and np.array_equal(mask, exp_mask), \
        "kernel specialized for the causal mask produced by setup_inputs()"

    flags = tuple(bool(np.any(np.asarray(x))) for x in (bq, bk, bv, bo))
    nc = _get_nc(flags)
    in_maps = _prep_inputs(q, k, v, Wq, bq, Wk, bk, Wv, bv, Wo, bo)
    res = run_bass_kernel_spmd(nc, in_maps, list(range(NCORES)),
                               trace=_trace)
    if _trace:
        _cache["last_result"] = res
    full = np.empty((B, S, D), np.float32)
    for c in range(NCORES):
        b, hf = divmod(c, 2)
        full[b, :, hf * HH:(hf + 1) * HH] = res.results[c]["out"]
    return full
